# revision 4
# baseline (speedup 1.0000x reference)
"""DTCWT forward (J=3) for x:[16,3,512,512] f32 on 8 Trainium2 NeuronCores.

Strategy: pure batch data-parallel — 2 samples (6 images of 512x512) per core.
Every 1-D filtering stage of the transform (level-1 odd-tap col/row filters,
level-2/3 q-shift decimated filters, incl. symmetric extension and the
tree-interleave) is a dense operator matrix precomputed on host, and executed
on the tensor engine as fp32r matmuls with the *image data as the stationary
operand* — so each stage's output comes out transposed relative to its input,
which makes the H-stage/W-stage pairs chain with zero explicit transposes.
Outputs of the H-stages are split into even/odd row planes (matrix rows
pre-split on host) so that the q2c quad->complex butterflies need only
free-axis (stride-2) access, done on the vector engine.
"""

import sys

for _p in ("/opt/trn_rl_repo", "/root/.axon_site/_ro/trn_rl_repo"):
    if _p not in sys.path:
        sys.path.append(_p)

from contextlib import ExitStack

import numpy as np

import concourse.bacc as bacc
import concourse.tile as tile
from concourse import mybir
from concourse.bass_utils import run_bass_kernel_spmd

F32 = mybir.dt.float32
F32R = mybir.dt.float32r
SQRT1_2 = 0.7071067811865476

N_CORES = 8
NS = 2          # samples per core
NC_CH = 3       # channels
N_IMG = NS * NC_CH


# ----------------------------------------------------------------------------
# Host-side operator matrices
# ----------------------------------------------------------------------------

def _refl(idx, n):
    idx = np.asarray(idx)
    idx = np.mod(idx, 2 * n)
    return np.where(idx < n, idx, 2 * n - 1 - idx)


def colfilter_mat(h, n):
    h = np.asarray(h, dtype=np.float64)
    L = h.shape[0]
    m = L // 2
    A = np.zeros((n, n), dtype=np.float64)
    for i in range(n):
        for k in range(L):
            A[i, int(_refl(i + k - m, n))] += h[k]
    return A


def dfilt_mat(fa, fb, n, highpass):
    fa = np.asarray(fa, dtype=np.float64)
    fb = np.asarray(fb, dtype=np.float64)
    m = fa.shape[0]
    farev, fbrev = fa[::-1], fb[::-1]
    n_q = n // 4
    Da = np.zeros((n_q, n), dtype=np.float64)
    Db = np.zeros((n_q, n), dtype=np.float64)
    for i in range(n_q):
        for k in range(m):
            Da[i, int(_refl(4 * i + 2 + 2 * k - m, n))] += farev[k]
            Db[i, int(_refl(4 * i + 3 + 2 * k - m, n))] += fbrev[k]
    first, second = (Db, Da) if highpass else (Da, Db)
    D = np.zeros((n // 2, n), dtype=np.float64)
    D[0::2] = first
    D[1::2] = second
    return D


def build_host_mats(h0o, h1o, h0a, h0b, h1a, h1b):
    C5 = colfilter_mat(h0o, 512)
    C7 = colfilter_mat(h1o, 512)
    Dlo = dfilt_mat(h0b, h0a, 512, False)
    Dhi = dfilt_mat(h1b, h1a, 512, True)
    Dlo3 = dfilt_mat(h0b, h0a, 256, False)
    Dhi3 = dfilt_mat(h1b, h1a, 256, True)

    f32 = lambda a: np.ascontiguousarray(a, dtype=np.float32)

    # L1 stage A moving: per H-block j, cols [A5e|A5o|A7e|A7o] (256 each)
    MA = np.zeros((128, 4, 1024), dtype=np.float64)
    for j in range(4):
        sl = slice(128 * j, 128 * j + 128)
        MA[:, j, 0:256] = C5[0::2][:, sl].T
        MA[:, j, 256:512] = C5[1::2][:, sl].T
        MA[:, j, 512:768] = C7[0::2][:, sl].T
        MA[:, j, 768:1024] = C7[1::2][:, sl].T

    # L1 stage B moving: per W-block j, cols [C5 | C7] (512 each)
    MB = np.zeros((128, 4, 1024), dtype=np.float64)
    for j in range(4):
        sl = slice(128 * j, 128 * j + 128)
        MB[:, j, 0:512] = C5[:, sl].T
        MB[:, j, 512:1024] = C7[:, sl].T

    # L2 stage A moving: per input block idx=(par,hc), cols
    # [Dlo_e|Dlo_o|Dhi_e|Dhi_o] (128 each)
    Dcat = np.vstack([Dlo[0::2], Dlo[1::2], Dhi[0::2], Dhi[1::2]])  # [512,512]
    ML2 = np.zeros((128, 4, 512), dtype=np.float64)
    for idx in range(4):
        par, hc = divmod(idx, 2)
        cols = 2 * (128 * hc + np.arange(128)) + par
        ML2[:, idx, :] = Dcat[:, cols].T

    # L2 stage B moving: per W-block j, cols [Dlo | Dhi] (256 each)
    Dcat2 = np.vstack([Dlo, Dhi])  # [512, 512]
    ML2B = np.zeros((128, 4, 512), dtype=np.float64)
    for j in range(4):
        ML2B[:, j, :] = Dcat2[:, 128 * j:128 * j + 128].T

    # L3 stage A moving: per input parity, cols [D3lo_e|D3lo_o|D3hi_e|D3hi_o]
    D3cat = np.vstack([Dlo3[0::2], Dlo3[1::2], Dhi3[0::2], Dhi3[1::2]])  # [256,256]
    ML3 = np.zeros((128, 2, 256), dtype=np.float64)
    for par in range(2):
        cols = 2 * np.arange(128) + par
        ML3[:, par, :] = D3cat[:, cols].T

    # L3 stage B moving: per W2-block m, cols [D3lo | D3hi] (128 each)
    D3cat2 = np.vstack([Dlo3, Dhi3])  # [256, 256]
    ML3B = np.zeros((128, 2, 256), dtype=np.float64)
    for m in range(2):
        ML3B[:, m, :] = D3cat2[:, 128 * m:128 * m + 128].T

    return {"MA": f32(MA), "MB": f32(MB), "ML2": f32(ML2),
            "ML2B": f32(ML2B), "ML3": f32(ML3), "ML3B": f32(ML3B)}


# ----------------------------------------------------------------------------
# Bass program (one core's work: [NS, 3, 512, 512] -> outputs)
# ----------------------------------------------------------------------------

def build_nc():
    nc = bacc.Bacc("TRN2", target_bir_lowering=False)

    x = nc.dram_tensor("x", [NS, NC_CH, 512, 512], F32R, kind="ExternalInput")
    dMA = nc.dram_tensor("MA", [128, 4, 1024], F32R, kind="ExternalInput")
    dMB = nc.dram_tensor("MB", [128, 4, 1024], F32R, kind="ExternalInput")
    dML2 = nc.dram_tensor("ML2", [128, 4, 512], F32R, kind="ExternalInput")
    dML2B = nc.dram_tensor("ML2B", [128, 4, 512], F32R, kind="ExternalInput")
    dML3 = nc.dram_tensor("ML3", [128, 2, 256], F32R, kind="ExternalInput")
    dML3B = nc.dram_tensor("ML3B", [128, 2, 256], F32R, kind="ExternalInput")

    yl = nc.dram_tensor("yl", [NS, NC_CH, 128, 128], F32, kind="ExternalOutput")
    yh1 = nc.dram_tensor("yh1", [NS, NC_CH, 6, 256, 256, 2], F32, kind="ExternalOutput")
    yh2 = nc.dram_tensor("yh2", [NS, NC_CH, 6, 128, 128, 2], F32, kind="ExternalOutput")
    yh3 = nc.dram_tensor("yh3", [NS, NC_CH, 6, 64, 64, 2], F32, kind="ExternalOutput")

    # orientation slots (z1, z2) per filter pair
    SLOT = {"p05": (0, 5), "p14": (1, 4), "p23": (2, 3)}
    ORI = ["p05", "p23", "p14"]

    with tile.TileContext(nc) as tc:
        with ExitStack() as ctx:
            mats = ctx.enter_context(tc.tile_pool(name="mats", bufs=1))
            xp = ctx.enter_context(tc.tile_pool(name="xp", bufs=2))
            ltp = ctx.enter_context(tc.tile_pool(name="ltp", bufs=2))
            lolop = ctx.enter_context(tc.tile_pool(name="lolop", bufs=2))
            yep = ctx.enter_context(tc.tile_pool(name="yep", bufs=1))
            zp = ctx.enter_context(tc.tile_pool(name="zp", bufs=4))
            l2p = ctx.enter_context(tc.tile_pool(name="l2p", bufs=2))
            l3p = ctx.enter_context(tc.tile_pool(name="l3p", bufs=2))
            psp = ctx.enter_context(tc.tile_pool(name="psp", bufs=6, space="PSUM"))

            sMA = mats.tile([128, 4, 1024], F32R)
            nc.sync.dma_start(out=sMA, in_=dMA[:, :, :])
            sMB = mats.tile([128, 4, 1024], F32R)
            nc.sync.dma_start(out=sMB, in_=dMB[:, :, :])
            sML2 = mats.tile([128, 4, 512], F32R)
            nc.sync.dma_start(out=sML2, in_=dML2[:, :, :])
            sML2B = mats.tile([128, 4, 512], F32R)
            nc.sync.dma_start(out=sML2B, in_=dML2B[:, :, :])
            sML3 = mats.tile([128, 2, 256], F32R)
            nc.sync.dma_start(out=sML3, in_=dML3[:, :, :])
            sML3B = mats.tile([128, 2, 256], F32R)
            nc.sync.dma_start(out=sML3B, in_=dML3B[:, :, :])

            for n in range(NS):
                for ch in range(NC_CH):
                    img_kernel(nc, tc, x, yl, yh1, yh2, yh3, n, ch,
                               sMA, sMB, sML2, sML2B, sML3, sML3B,
                               xp, ltp, lolop, yep, zp, l2p, l3p, psp,
                               SLOT, ORI)
    nc.compile()
    return nc


def img_kernel(nc, tc, x, yl, yh1, yh2, yh3, n, ch,
               sMA, sMB, sML2, sML2B, sML3, sML3B,
               xp, ltp, lolop, yep, zp, l2p, l3p, psp, SLOT, ORI):
    s = SQRT1_2

    # ---- load image: [128 part, 4 hblk, 512 w]
    xt = xp.tile([128, 4, 512], F32R, tag="xt")
    nc.sync.dma_start(out=xt, in_=x[n, ch].rearrange("(t p) w -> p t w", p=128))

    # ---- L1 stage A: loT/hiT parity planes, [128 part(W chunk), 4 m, 256 H/2]
    loT_e = ltp.tile([128, 4, 256], F32R, tag="loT_e")
    loT_o = ltp.tile([128, 4, 256], F32R, tag="loT_o")
    hiT_e = ltp.tile([128, 4, 256], F32R, tag="hiT_e")
    hiT_o = ltp.tile([128, 4, 256], F32R, tag="hiT_o")
    for m in range(4):
        ps_lo = psp.tile([128, 512], F32, tag="ps")
        ps_hi = psp.tile([128, 512], F32, tag="ps")
        for j in range(4):
            st = xt[:, j, 128 * m:128 * m + 128]
            nc.tensor.matmul(ps_lo, st, sMA[:, j, 0:512],
                             start=(j == 0), stop=(j == 3))
            nc.tensor.matmul(ps_hi, st, sMA[:, j, 512:1024],
                             start=(j == 0), stop=(j == 3))
        nc.scalar.copy(loT_e[:, m, :], ps_lo[:, 0:256])
        nc.scalar.copy(loT_o[:, m, :], ps_lo[:, 256:512])
        nc.scalar.copy(hiT_e[:, m, :], ps_hi[:, 0:256])
        nc.scalar.copy(hiT_o[:, m, :], ps_hi[:, 256:512])

    # ---- L1 stage B + q2c per parity pair
    lolo_e = lolop.tile([128, 2, 512], F32R, tag="lolo_e")
    lolo_o = lolop.tile([128, 2, 512], F32R, tag="lolo_o")
    YE = {o: yep.tile([128, 2, 512], F32, tag="YE_" + o, name="YE_" + o) for o in ORI}
    YO = {o: yep.tile([128, 2, 512], F32, tag="YO_" + o, name="YO_" + o) for o in ORI}

    for src, lolo_dst, d1, d2 in (
        (loT_e, lolo_e, None, YE["p23"]),
        (loT_o, lolo_o, None, YO["p23"]),
        (hiT_e, None, YE["p05"], YE["p14"]),
        (hiT_o, None, YO["p05"], YO["p14"]),
    ):
        for hc in range(2):
            ps1 = psp.tile([128, 512], F32, tag="ps")
            ps2 = psp.tile([128, 512], F32, tag="ps")
            for j in range(4):
                st = src[:, j, 128 * hc:128 * hc + 128]
                nc.tensor.matmul(ps1, st, sMB[:, j, 0:512],
                                 start=(j == 0), stop=(j == 3))
                nc.tensor.matmul(ps2, st, sMB[:, j, 512:1024],
                                 start=(j == 0), stop=(j == 3))
            if lolo_dst is not None:
                nc.scalar.copy(lolo_dst[:, hc, :], ps1)
            else:
                nc.scalar.mul(d1[:, hc, :], ps1, s)
            nc.scalar.mul(d2[:, hc, :], ps2, s)

    # q2c L1 -> yh1
    for o in ORI:
        s1, s2 = SLOT[o]
        dst1 = yh1[n, ch, s1].rearrange("(t p) w r -> t p (w r)", p=128)
        dst2 = yh1[n, ch, s2].rearrange("(t p) w r -> t p (w r)", p=128)
        for hc in range(2):
            a = YE[o][:, hc, 0::2]
            b = YE[o][:, hc, 1::2]
            c = YO[o][:, hc, 0::2]
            d = YO[o][:, hc, 1::2]
            z1 = zp.tile([128, 512], F32, tag="z")
            z2 = zp.tile([128, 512], F32, tag="z")
            nc.vector.tensor_sub(z1[:, 0::2], a, d)
            nc.vector.tensor_add(z1[:, 1::2], b, c)
            nc.vector.tensor_add(z2[:, 0::2], a, d)
            nc.vector.tensor_sub(z2[:, 1::2], b, c)
            nc.sync.dma_start(out=dst1[hc], in_=z1)
            nc.sync.dma_start(out=dst2[hc], in_=z2)

    # ---- L2 stage A: loT2/hiT2 [128 part(W chunk), 4 m, 128 H2par]
    loT2_e = l2p.tile([128, 4, 128], F32R, tag="loT2_e")
    loT2_o = l2p.tile([128, 4, 128], F32R, tag="loT2_o")
    hiT2_e = l2p.tile([128, 4, 128], F32R, tag="hiT2_e")
    hiT2_o = l2p.tile([128, 4, 128], F32R, tag="hiT2_o")
    for m in range(4):
        ps = psp.tile([128, 512], F32, tag="ps")
        for idx in range(4):
            par, hc = divmod(idx, 2)
            src = lolo_e if par == 0 else lolo_o
            nc.tensor.matmul(ps, src[:, hc, 128 * m:128 * m + 128],
                             sML2[:, idx, :], start=(idx == 0), stop=(idx == 3))
        nc.scalar.copy(loT2_e[:, m, :], ps[:, 0:128])
        nc.scalar.copy(loT2_o[:, m, :], ps[:, 128:256])
        nc.scalar.copy(hiT2_e[:, m, :], ps[:, 256:384])
        nc.scalar.copy(hiT2_o[:, m, :], ps[:, 384:512])

    # ---- L2 stage B
    lolo2_e = l2p.tile([128, 256], F32R, tag="lolo2_e")
    lolo2_o = l2p.tile([128, 256], F32R, tag="lolo2_o")
    Y2E = {o: l2p.tile([128, 256], F32, tag="Y2E_" + o, name="Y2E_" + o) for o in ORI}
    Y2O = {o: l2p.tile([128, 256], F32, tag="Y2O_" + o, name="Y2O_" + o) for o in ORI}
    for src, lolo_dst, d1, d2 in (
        (loT2_e, lolo2_e, None, Y2E["p23"]),
        (loT2_o, lolo2_o, None, Y2O["p23"]),
        (hiT2_e, None, Y2E["p05"], Y2E["p14"]),
        (hiT2_o, None, Y2O["p05"], Y2O["p14"]),
    ):
        ps = psp.tile([128, 512], F32, tag="ps")
        for j in range(4):
            nc.tensor.matmul(ps, src[:, j, :], sML2B[:, j, :],
                             start=(j == 0), stop=(j == 3))
        if lolo_dst is not None:
            nc.scalar.copy(lolo_dst, ps[:, 0:256])
        else:
            nc.scalar.mul(d1, ps[:, 0:256], s)
        nc.scalar.mul(d2, ps[:, 256:512], s)

    # q2c L2 -> yh2
    for o in ORI:
        s1, s2 = SLOT[o]
        a = Y2E[o][:, 0::2]
        b = Y2E[o][:, 1::2]
        c = Y2O[o][:, 0::2]
        d = Y2O[o][:, 1::2]
        z1 = zp.tile([128, 256], F32, tag="z2")
        z2 = zp.tile([128, 256], F32, tag="z2")
        nc.vector.tensor_sub(z1[:, 0::2], a, d)
        nc.vector.tensor_add(z1[:, 1::2], b, c)
        nc.vector.tensor_add(z2[:, 0::2], a, d)
        nc.vector.tensor_sub(z2[:, 1::2], b, c)
        nc.sync.dma_start(out=yh2[n, ch, s1].rearrange("p w r -> p (w r)"), in_=z1)
        nc.sync.dma_start(out=yh2[n, ch, s2].rearrange("p w r -> p (w r)"), in_=z2)

    # ---- L3 stage A: loT3/hiT3 [128 part(W2 chunk), 2 m, 64 H3par]
    loT3_e = l3p.tile([128, 2, 64], F32R, tag="loT3_e")
    loT3_o = l3p.tile([128, 2, 64], F32R, tag="loT3_o")
    hiT3_e = l3p.tile([128, 2, 64], F32R, tag="hiT3_e")
    hiT3_o = l3p.tile([128, 2, 64], F32R, tag="hiT3_o")
    for m in range(2):
        ps = psp.tile([128, 256], F32, tag="ps3", bufs=2)
        for par in range(2):
            src = lolo2_e if par == 0 else lolo2_o
            nc.tensor.matmul(ps, src[:, 128 * m:128 * m + 128],
                             sML3[:, par, :], start=(par == 0), stop=(par == 1))
        nc.scalar.copy(loT3_e[:, m, :], ps[:, 0:64])
        nc.scalar.copy(loT3_o[:, m, :], ps[:, 64:128])
        nc.scalar.copy(hiT3_e[:, m, :], ps[:, 128:192])
        nc.scalar.copy(hiT3_o[:, m, :], ps[:, 192:256])

    # ---- L3 stage B
    yl_ap = yl[n, ch].rearrange("(h par) w -> par h w", par=2)
    Y3E = {o: l3p.tile([64, 128], F32, tag="Y3E_" + o, name="Y3E_" + o) for o in ORI}
    Y3O = {o: l3p.tile([64, 128], F32, tag="Y3O_" + o, name="Y3O_" + o) for o in ORI}
    for src, is_lo, par, d1, d2 in (
        (loT3_e, True, 0, None, Y3E["p23"]),
        (loT3_o, True, 1, None, Y3O["p23"]),
        (hiT3_e, False, 0, Y3E["p05"], Y3E["p14"]),
        (hiT3_o, False, 1, Y3O["p05"], Y3O["p14"]),
    ):
        ps = psp.tile([64, 256], F32, tag="ps3", bufs=2)
        for m in range(2):
            nc.tensor.matmul(ps, src[:, m, :], sML3B[:, m, :],
                             start=(m == 0), stop=(m == 1))
        if is_lo:
            lolo3 = l3p.tile([64, 128], F32, tag="lolo3_%d" % par)
            nc.scalar.copy(lolo3, ps[:, 0:128])
            nc.sync.dma_start(out=yl_ap[par], in_=lolo3)
            nc.scalar.mul(d2, ps[:, 128:256], s)
        else:
            nc.scalar.mul(d1, ps[:, 0:128], s)
            nc.scalar.mul(d2, ps[:, 128:256], s)

    # q2c L3 -> yh3
    for o in ORI:
        s1, s2 = SLOT[o]
        a = Y3E[o][:, 0::2]
        b = Y3E[o][:, 1::2]
        c = Y3O[o][:, 0::2]
        d = Y3O[o][:, 1::2]
        z1 = zp.tile([64, 128], F32, tag="z3")
        z2 = zp.tile([64, 128], F32, tag="z3")
        nc.vector.tensor_sub(z1[:, 0::2], a, d)
        nc.vector.tensor_add(z1[:, 1::2], b, c)
        nc.vector.tensor_add(z2[:, 0::2], a, d)
        nc.vector.tensor_sub(z2[:, 1::2], b, c)
        nc.sync.dma_start(out=yh3[n, ch, s1].rearrange("p w r -> p (w r)"), in_=z1)
        nc.sync.dma_start(out=yh3[n, ch, s2].rearrange("p w r -> p (w r)"), in_=z2)


# ----------------------------------------------------------------------------
# Public entry point
# ----------------------------------------------------------------------------

_CACHE = {}


def _get_nc():
    if "nc" not in _CACHE:
        _CACHE["nc"] = build_nc()
    return _CACHE["nc"]


def kernel(x, h0o, h1o, h0a, h0b, h1a, h1b, _trace=False):
    x = np.ascontiguousarray(np.asarray(x), dtype=np.float32)
    mats = build_host_mats(np.asarray(h0o), np.asarray(h1o), np.asarray(h0a),
                           np.asarray(h0b), np.asarray(h1a), np.asarray(h1b))
    nc = _get_nc()

    in_maps = []
    for c in range(N_CORES):
        m = {"x": np.ascontiguousarray(x[c * NS:(c + 1) * NS])}
        m.update(mats)
        in_maps.append(m)

    res = run_bass_kernel_spmd(nc, in_maps, core_ids=list(range(N_CORES)))

    yl = np.concatenate([res.results[c]["yl"] for c in range(N_CORES)], axis=0)
    yh1 = np.concatenate([res.results[c]["yh1"] for c in range(N_CORES)], axis=0)
    yh2 = np.concatenate([res.results[c]["yh2"] for c in range(N_CORES)], axis=0)
    yh3 = np.concatenate([res.results[c]["yh3"] for c in range(N_CORES)], axis=0)
    if _trace:
        return (yl, yh1, yh2, yh3), res
    return yl, yh1, yh2, yh3


# revision 12
# speedup vs baseline: 2556.7772x; 2556.7772x over previous
"""DTCWT forward (J=3) for x:[16,3,512,512] f32 on 8 Trainium2 NeuronCores.

Batch data-parallel: 2 samples (6 images) per core. Every 1-D filtering
stage (level-1 odd-tap col/row filters, level-2/3 q-shift decimated
filters, incl. symmetric extension + tree interleave + sqrt(1/2) scaling)
is a host-precomputed operator matrix, run on the tensor engine as fp32r
matmuls with the image data as the stationary operand, so each stage's
output is transposed relative to its input and H/W stage pairs chain with
no explicit transposes. Moving operands are *windowed* to the band support
(padded to >=256 free for full-rate fp32r). H-stage outputs are split into
even/odd row planes so the q2c butterflies are free-axis stride-2 vector
ops, reading the even-plane straight out of PSUM.
"""

import sys

for _p in ("/opt/trn_rl_repo", "/root/.axon_site/_ro/trn_rl_repo"):
    if _p not in sys.path:
        sys.path.append(_p)

from contextlib import ExitStack

import numpy as np

import concourse.bacc as bacc
import concourse.tile as tile
from concourse import mybir
from concourse.bass_utils import run_bass_kernel_spmd

F32 = mybir.dt.float32
F32R = mybir.dt.float32r
SQRT1_2 = 0.7071067811865476

N_CORES = 8
NS = 2
NC_CH = 3

# orientation slots (z1, z2) per filter pair
SLOT = {"p05": (0, 5), "p14": (1, 4), "p23": (2, 3)}
ORI = ["p05", "p23", "p14"]


# ----------------------------------------------------------------------------
# Host-side operator matrices
# ----------------------------------------------------------------------------

def _refl(idx, n):
    idx = np.asarray(idx)
    idx = np.mod(idx, 2 * n)
    return np.where(idx < n, idx, 2 * n - 1 - idx)


def colfilter_mat(h, n):
    h = np.asarray(h, dtype=np.float64)
    L = h.shape[0]
    m = L // 2
    A = np.zeros((n, n), dtype=np.float64)
    for i in range(n):
        for k in range(L):
            A[i, int(_refl(i + k - m, n))] += h[k]
    return A


def dfilt_mat(fa, fb, n, highpass):
    fa = np.asarray(fa, dtype=np.float64)
    fb = np.asarray(fb, dtype=np.float64)
    m = fa.shape[0]
    farev, fbrev = fa[::-1], fb[::-1]
    n_q = n // 4
    Da = np.zeros((n_q, n), dtype=np.float64)
    Db = np.zeros((n_q, n), dtype=np.float64)
    for i in range(n_q):
        for k in range(m):
            Da[i, int(_refl(4 * i + 2 + 2 * k - m, n))] += farev[k]
            Db[i, int(_refl(4 * i + 3 + 2 * k - m, n))] += fbrev[k]
    first, second = (Db, Da) if highpass else (Da, Db)
    D = np.zeros((n // 2, n), dtype=np.float64)
    D[0::2] = first
    D[1::2] = second
    return D


def _allmats(h0o, h1o, h0a, h0b, h1a, h1b):
    return {
        "C5": colfilter_mat(h0o, 512), "C7": colfilter_mat(h1o, 512),
        "Dlo": dfilt_mat(h0b, h0a, 512, False),
        "Dhi": dfilt_mat(h1b, h1a, 512, True),
        "Dlo3": dfilt_mat(h0b, h0a, 256, False),
        "Dhi3": dfilt_mat(h1b, h1a, 256, True),
    }


def _nzrange(M2):
    nz = np.where(np.abs(M2).sum(axis=0) > 0)[0]
    return int(nz[0]), int(nz[-1]) + 1


def _fitwin(lo, hi, width, limit):
    """Even-aligned window [s, s+width) within [0, limit] covering [lo, hi)."""
    assert hi - lo <= width, (lo, hi, width)
    s = max(0, hi - width)
    s += s % 2  # round up to even so the window still covers hi
    s = min(s, limit - width)
    assert s <= lo and s + width >= hi and 0 <= s <= limit - width and s % 2 == 0
    return s


def build_plan(mats):
    """Window metadata + packed (windowed, scale-folded) moving tensors."""
    s = SQRT1_2
    C5, C7 = mats["C5"], mats["C7"]
    Dlo, Dhi = mats["Dlo"], mats["Dhi"]
    Dlo3, Dhi3 = mats["Dlo3"], mats["Dhi3"]
    W = {}
    P = {}

    # L1-A: per (j, filter f): [Cf_e | Cf_o] cols, shared window width 128
    MA = np.zeros((128, 4, 2, 2, 128), dtype=np.float64)  # [k, j, f, par, x]
    W["A1"] = []
    cov = np.zeros((2, 2, 256), dtype=bool)
    for j in range(4):
        ws = []
        for f, C in enumerate((C5, C7)):
            lo_e, hi_e = _nzrange(C[0::2][:, 128 * j:128 * j + 128].T)
            lo_o, hi_o = _nzrange(C[1::2][:, 128 * j:128 * j + 128].T)
            w = _fitwin(min(lo_e, lo_o), max(hi_e, hi_o), 128, 256)
            ws.append(w)
            for par, Cp in enumerate((C[0::2], C[1::2])):
                MA[:, j, f, par, :] = Cp[w:w + 128, 128 * j:128 * j + 128].T
                cov[f, par, w:w + 128] = True
        W["A1"].append(ws)
    assert cov.all()
    P["MA"] = MA.reshape(128, 4, 512)

    # L1-B: per j: window 256 over 512; variants [C5, s*C5, s*C7]
    MB = np.zeros((128, 4, 3, 256), dtype=np.float64)
    W["B1"] = []
    cov = np.zeros(512, dtype=bool)
    for j in range(4):
        l1, h1 = _nzrange(C5[:, 128 * j:128 * j + 128].T)
        l2, h2 = _nzrange(C7[:, 128 * j:128 * j + 128].T)
        w = _fitwin(min(l1, l2), max(h1, h2), 256, 512)
        W["B1"].append(w)
        MB[:, j, 0, :] = C5[w:w + 256, 128 * j:128 * j + 128].T
        MB[:, j, 1, :] = s * C5[w:w + 256, 128 * j:128 * j + 128].T
        MB[:, j, 2, :] = s * C7[w:w + 256, 128 * j:128 * j + 128].T
        cov[w:w + 256] = True
    assert cov.all()
    P["MB"] = MB.reshape(128, 4, 768)

    # L2-A: per idx=(par,hc): 4 quarters [Dlo_e|Dlo_o|Dhi_e|Dhi_o],
    # shared window width 72 within each 128-quarter
    Dq = [Dlo[0::2], Dlo[1::2], Dhi[0::2], Dhi[1::2]]
    ML2 = np.zeros((128, 4, 4, 72), dtype=np.float64)  # [k, idx, q, x]
    W["A2"] = []
    cov = np.zeros((4, 128), dtype=bool)
    for idx in range(4):
        par, hc = divmod(idx, 2)
        cols = 2 * (128 * hc + np.arange(128)) + par
        rngs = [_nzrange(D[:, cols].T) for D in Dq]
        w = _fitwin(min(r[0] for r in rngs), max(r[1] for r in rngs), 72, 128)
        W["A2"].append(w)
        for q, D in enumerate(Dq):
            ML2[:, idx, q, :] = D[w:w + 72, :][:, cols].T
            cov[q, w:w + 72] = True
    assert cov.all()
    P["ML2"] = ML2.reshape(128, 4, 288)

    # L2-B: per (j, variant v): halves [lo_scale*Dlo | s*Dhi], window 128/half
    ML2B = np.zeros((128, 4, 2, 2, 128), dtype=np.float64)  # [k, j, v, h, x]
    W["B2"] = []
    cov = np.zeros(256, dtype=bool)
    for j in range(4):
        r1 = _nzrange(Dlo[:, 128 * j:128 * j + 128].T)
        r2 = _nzrange(Dhi[:, 128 * j:128 * j + 128].T)
        w = _fitwin(min(r1[0], r2[0]), max(r1[1], r2[1]), 128, 256)
        W["B2"].append(w)
        for v, lo_scale in enumerate((1.0, s)):
            ML2B[:, j, v, 0, :] = lo_scale * Dlo[w:w + 128, 128 * j:128 * j + 128].T
            ML2B[:, j, v, 1, :] = s * Dhi[w:w + 128, 128 * j:128 * j + 128].T
        cov[w:w + 128] = True
    assert cov.all()
    P["ML2B"] = ML2B.reshape(128, 4, 512)

    # L3-A: dense, per input parity
    D3cat = np.vstack([Dlo3[0::2], Dlo3[1::2], Dhi3[0::2], Dhi3[1::2]])
    ML3 = np.zeros((128, 2, 256), dtype=np.float64)
    for par in range(2):
        cols = 2 * np.arange(128) + par
        ML3[:, par, :] = D3cat[:, cols].T
    P["ML3"] = ML3

    # L3-B: dense, per (m, variant)
    ML3B = np.zeros((128, 2, 2, 256), dtype=np.float64)
    for m in range(2):
        sl = slice(128 * m, 128 * m + 128)
        for v, lo_scale in enumerate((1.0, s)):
            ML3B[:, m, v, 0:128] = lo_scale * Dlo3[:, sl].T
            ML3B[:, m, v, 128:256] = s * Dhi3[:, sl].T
    P["ML3B"] = ML3B.reshape(128, 2, 512)

    P = {k: np.ascontiguousarray(v, dtype=np.float32) for k, v in P.items()}
    return W, P


def structural_windows():
    o = np.ones
    return build_plan(_allmats(o(5), o(7), o(10), o(10), o(10), o(10)))[0]


def build_host_mats(h0o, h1o, h0a, h0b, h1a, h1b):
    return build_plan(_allmats(h0o, h1o, h0a, h0b, h1a, h1b))[1]


# ----------------------------------------------------------------------------
# Bass program
# ----------------------------------------------------------------------------

def build_nc(repeat=1):
    Wn = structural_windows()
    nc = bacc.Bacc("TRN2", target_bir_lowering=False)

    x = nc.dram_tensor("x", [NS, NC_CH, 512, 512], F32R, kind="ExternalInput")
    dMA = nc.dram_tensor("MA", [128, 4, 512], F32R, kind="ExternalInput")
    dMB = nc.dram_tensor("MB", [128, 4, 768], F32R, kind="ExternalInput")
    dML2 = nc.dram_tensor("ML2", [128, 4, 288], F32R, kind="ExternalInput")
    dML2B = nc.dram_tensor("ML2B", [128, 4, 512], F32R, kind="ExternalInput")
    dML3 = nc.dram_tensor("ML3", [128, 2, 256], F32R, kind="ExternalInput")
    dML3B = nc.dram_tensor("ML3B", [128, 2, 512], F32R, kind="ExternalInput")

    yl = nc.dram_tensor("yl", [NS, NC_CH, 128, 128], F32, kind="ExternalOutput")
    yh1 = nc.dram_tensor("yh1", [NS, NC_CH, 6, 256, 256, 2], F32, kind="ExternalOutput")
    yh2 = nc.dram_tensor("yh2", [NS, NC_CH, 6, 128, 128, 2], F32, kind="ExternalOutput")
    yh3 = nc.dram_tensor("yh3", [NS, NC_CH, 6, 64, 64, 2], F32, kind="ExternalOutput")

    with tile.TileContext(nc) as tc:
        with ExitStack() as ctx:
            mp = ctx.enter_context(tc.tile_pool(name="mp", bufs=1))
            xp = ctx.enter_context(tc.tile_pool(name="xp", bufs=2))
            ltp = ctx.enter_context(tc.tile_pool(name="ltp", bufs=2))
            lolop = ctx.enter_context(tc.tile_pool(name="lolop", bufs=2))
            yop = ctx.enter_context(tc.tile_pool(name="yop", bufs=2))
            zp = ctx.enter_context(tc.tile_pool(name="zp", bufs=6))
            l2p = ctx.enter_context(tc.tile_pool(name="l2p", bufs=2))
            l3p = ctx.enter_context(tc.tile_pool(name="l3p", bufs=2))
            psp = ctx.enter_context(tc.tile_pool(name="psp", bufs=8, space="PSUM"))

            sMA = mp.tile([128, 4, 512], F32R)
            nc.sync.dma_start(out=sMA, in_=dMA[:, :, :])
            sMB = mp.tile([128, 4, 768], F32R)
            nc.sync.dma_start(out=sMB, in_=dMB[:, :, :])
            sML2 = mp.tile([128, 4, 288], F32R)
            nc.sync.dma_start(out=sML2, in_=dML2[:, :, :])
            sML2B = mp.tile([128, 4, 512], F32R)
            nc.sync.dma_start(out=sML2B, in_=dML2B[:, :, :])
            sML3 = mp.tile([128, 2, 256], F32R)
            nc.sync.dma_start(out=sML3, in_=dML3[:, :, :])
            sML3B = mp.tile([128, 2, 512], F32R)
            nc.sync.dma_start(out=sML3B, in_=dML3B[:, :, :])
            sm = dict(MA=sMA, MB=sMB, ML2=sML2, ML2B=sML2B, ML3=sML3,
                      ML3B=sML3B)
            pools = dict(xp=xp, ltp=ltp, lolop=lolop, yop=yop, zp=zp,
                         l2p=l2p, l3p=l3p, psp=psp)

            bal = EngBal(nc)

            def body():
                for n in range(NS):
                    for ch in range(NC_CH):
                        img_kernel(nc, x, yl, yh1, yh2, yh3, n, ch, sm,
                                   pools, Wn, bal)

            if repeat > 1:
                with tc.For_i(0, repeat, 1):
                    body()
            else:
                body()
    nc.compile()
    return nc


class EngBal:
    """Greedy ACT/DVE load balancer for PSUM->SBUF drains; q2c work is
    charged to DVE so drains flow to whichever engine is lighter."""

    def __init__(self, nc):
        self.nc = nc
        self.act = 0.0
        self.dve = 0.0

    def copy(self, out, in_, cost):
        if self.act <= self.dve:
            self.nc.scalar.copy(out, in_)
            self.act += cost
        else:
            self.nc.vector.tensor_copy(out, in_)
            self.dve += cost

    def q2c(self, zcats, o, E, O, w2, cost):
        """4 butterfly ops for orientation o into the cat z tiles."""
        nc = self.nc
        z1cat, z2cat = zcats
        s1, s2 = SLOT[o]
        a, b = E[:, 0::2], E[:, 1::2]
        c, d = O[:, 0::2], O[:, 1::2]
        z1 = z1cat[:, s1, :]
        z2 = z2cat[:, s2 - 3, :]
        nc.vector.tensor_sub(z1[:, 0::2], a, d)
        nc.vector.tensor_add(z1[:, 1::2], b, c)
        nc.vector.tensor_add(z2[:, 0::2], a, d)
        nc.vector.tensor_sub(z2[:, 1::2], b, c)
        self.dve += 4 * cost


def img_kernel(nc, x, yl, yh1, yh2, yh3, n, ch, sm, pools, Wn, bal):
    xp, ltp, lolop, yop, zp, l2p, l3p, psp = (
        pools["xp"], pools["ltp"], pools["lolop"], pools["yop"], pools["zp"],
        pools["l2p"], pools["l3p"], pools["psp"])

    xt = xp.tile([128, 4, 512], F32R, tag="xt")
    nc.sync.dma_start(out=xt, in_=x[n, ch].rearrange("(t p) w -> p t w", p=128))

    # ---- L1-A -> loT/hiT [128 (W), 4 wblk, 512=(e:256|o:256)]
    loT = ltp.tile([128, 4, 512], F32R, tag="loT")
    hiT = ltp.tile([128, 4, 512], F32R, tag="hiT")
    for m in range(4):
        ps_lo = psp.tile([128, 512], F32, tag="ps", bufs=8)
        ps_hi = psp.tile([128, 512], F32, tag="ps", bufs=8)
        plo = ps_lo.rearrange("p (c x) -> p c x", c=2)
        phi = ps_hi.rearrange("p (c x) -> p c x", c=2)
        for j in range(4):
            st = xt[:, j, 128 * m:128 * m + 128]
            w0 = Wn["A1"][j][0]
            w1 = Wn["A1"][j][1]
            nc.tensor.matmul(plo[:, :, w0:w0 + 128], st,
                             sm["MA"][:, j, 0:256],
                             start=(j == 0), stop=(j == 3))
            nc.tensor.matmul(phi[:, :, w1:w1 + 128], st,
                             sm["MA"][:, j, 256:512],
                             start=(j == 0), stop=(j == 3))
        bal.copy(loT[:, m, :], ps_lo, 730)
        bal.copy(hiT[:, m, :], ps_hi, 730)

    # ---- L1-B (+ q2c L1) per hc
    lolo_e = lolop.tile([128, 2, 512], F32R, tag="lolo_e")
    lolo_o = lolop.tile([128, 2, 512], F32R, tag="lolo_o")
    for hc in range(2):
        psE = {}
        yo = {}
        for srcname, par in (("lo", 0), ("lo", 1), ("hi", 0), ("hi", 1)):
            src = loT if srcname == "lo" else hiT
            off = 256 * par + 128 * hc
            v1 = 0 if srcname == "lo" else 1
            ps1 = psp.tile([128, 512], F32, tag="ps", bufs=8)
            ps2 = psp.tile([128, 512], F32, tag="ps", bufs=8)
            for j in range(4):
                st = src[:, j, off:off + 128]
                w = Wn["B1"][j]
                nc.tensor.matmul(ps1[:, w:w + 256], st,
                                 sm["MB"][:, j, 256 * v1:256 * v1 + 256],
                                 start=(j == 0), stop=(j == 3))
                nc.tensor.matmul(ps2[:, w:w + 256], st,
                                 sm["MB"][:, j, 512:768],
                                 start=(j == 0), stop=(j == 3))
            if srcname == "lo":
                dst = lolo_e if par == 0 else lolo_o
                bal.copy(dst[:, hc, :], ps1, 730)
                if par == 0:
                    psE["p23"] = ps2
                else:
                    t = yop.tile([128, 512], F32, tag="yo", name="yo_p23", bufs=3)
                    bal.copy(t, ps2, 730)
                    yo["p23"] = t
            else:
                if par == 0:
                    psE["p05"] = ps1
                    psE["p14"] = ps2
                else:
                    t1 = yop.tile([128, 512], F32, tag="yo", name="yo_p05", bufs=3)
                    t2 = yop.tile([128, 512], F32, tag="yo", name="yo_p14", bufs=3)
                    bal.copy(t1, ps1, 730)
                    bal.copy(t2, ps2, 730)
                    yo["p05"] = t1
                    yo["p14"] = t2
        z1cat = zp.tile([128, 3, 512], F32, tag="z", name="z1cat")
        z2cat = zp.tile([128, 3, 512], F32, tag="z", name="z2cat")
        for o in ORI:
            bal.q2c((z1cat, z2cat), o, psE[o], yo[o], 256, 533)
        d1 = yh1[n, ch, 0:3].rearrange("o (t p) w r -> t p o (w r)", p=128)
        d2 = yh1[n, ch, 3:6].rearrange("o (t p) w r -> t p o (w r)", p=128)
        nc.sync.dma_start(out=d1[hc], in_=z1cat)
        nc.sync.dma_start(out=d2[hc], in_=z2cat)

    # ---- L2-A -> lt2 [128 (W), 4 wblk, 512=(lo_e|lo_o|hi_e|hi_o)]
    lt2 = l2p.tile([128, 4, 512], F32R, tag="lt2")
    for m in range(4):
        ps = psp.tile([128, 512], F32, tag="ps", bufs=8)
        pq = ps.rearrange("p (q x) -> p q x", q=4)
        for idx in range(4):
            par, hc = divmod(idx, 2)
            src = lolo_e if par == 0 else lolo_o
            w = Wn["A2"][idx]
            nc.tensor.matmul(pq[:, :, w:w + 72],
                             src[:, hc, 128 * m:128 * m + 128],
                             sm["ML2"][:, idx, :],
                             start=(idx == 0), stop=(idx == 3))
        bal.copy(lt2[:, m, :], ps, 730)

    # ---- L2-B (+ q2c L2)
    lolo2_e = l2p.tile([128, 256], F32R, tag="lolo2_e")
    lolo2_o = l2p.tile([128, 256], F32R, tag="lolo2_o")
    psE2 = {}
    yo2 = {}
    for srcname, par in (("lo", 0), ("lo", 1), ("hi", 0), ("hi", 1)):
        off = 256 * (0 if srcname == "lo" else 1) + 128 * par
        v = 0 if srcname == "lo" else 1
        ps = psp.tile([128, 512], F32, tag="ps", bufs=8)
        ph = ps.rearrange("p (h x) -> p h x", h=2)
        for j in range(4):
            w = Wn["B2"][j]
            nc.tensor.matmul(ph[:, :, w:w + 128], lt2[:, j, off:off + 128],
                             sm["ML2B"][:, j, 256 * v:256 * v + 256],
                             start=(j == 0), stop=(j == 3))
        if srcname == "lo":
            dst = lolo2_e if par == 0 else lolo2_o
            bal.copy(dst, ps[:, 0:256], 420)
            if par == 0:
                psE2["p23"] = ps[:, 256:512]
            else:
                t = yop.tile([128, 256], F32, tag="yo2", name="yo2_p23", bufs=3)
                bal.copy(t, ps[:, 256:512], 420)
                yo2["p23"] = t
        else:
            if par == 0:
                psE2["p05"] = ps[:, 0:256]
                psE2["p14"] = ps[:, 256:512]
            else:
                t1 = yop.tile([128, 256], F32, tag="yo2", name="yo2_p05", bufs=3)
                t2 = yop.tile([128, 256], F32, tag="yo2", name="yo2_p14", bufs=3)
                bal.copy(t1, ps[:, 0:256], 420)
                bal.copy(t2, ps[:, 256:512], 420)
                yo2["p05"] = t1
                yo2["p14"] = t2
    z1cat2 = zp.tile([128, 3, 256], F32, tag="z2c", name="z1cat2")
    z2cat2 = zp.tile([128, 3, 256], F32, tag="z2c", name="z2cat2")
    for o in ORI:
        bal.q2c((z1cat2, z2cat2), o, psE2[o], yo2[o], 128, 300)
    nc.sync.dma_start(out=yh2[n, ch, 0:3].rearrange("o p w r -> p o (w r)"),
                      in_=z1cat2)
    nc.sync.dma_start(out=yh2[n, ch, 3:6].rearrange("o p w r -> p o (w r)"),
                      in_=z2cat2)

    # ---- L3-A -> lt3 [128 (W2 chunk), 2 m, 256=(lo_e|lo_o|hi_e|hi_o) x 64]
    lt3 = l3p.tile([128, 2, 256], F32R, tag="lt3")
    for m in range(2):
        ps = psp.tile([128, 256], F32, tag="ps", bufs=8)
        for par in range(2):
            src = lolo2_e if par == 0 else lolo2_o
            nc.tensor.matmul(ps, src[:, 128 * m:128 * m + 128],
                             sm["ML3"][:, par, :],
                             start=(par == 0), stop=(par == 1))
        bal.copy(lt3[:, m, :], ps, 420)

    # ---- L3-B (+ q2c L3, yl)
    yl_ap = yl[n, ch].rearrange("(h par) w -> par h w", par=2)
    psE3 = {}
    yo3 = {}
    for srcname, par in (("lo", 0), ("lo", 1), ("hi", 0), ("hi", 1)):
        off = 128 * (0 if srcname == "lo" else 1) + 64 * par
        v = 0 if srcname == "lo" else 1
        ps = psp.tile([64, 256], F32, tag="ps", bufs=8)
        for m in range(2):
            nc.tensor.matmul(ps, lt3[:, m, off:off + 64],
                             sm["ML3B"][:, m, 256 * v:256 * v + 256],
                             start=(m == 0), stop=(m == 1))
        if srcname == "lo":
            t = l3p.tile([64, 128], F32, tag="lolo3", name="lolo3_%d" % par)
            bal.copy(t, ps[:, 0:128], 250)
            nc.sync.dma_start(out=yl_ap[par], in_=t)
            if par == 0:
                psE3["p23"] = ps[:, 128:256]
            else:
                t2 = yop.tile([64, 128], F32, tag="yo3", name="yo3_p23", bufs=3)
                bal.copy(t2, ps[:, 128:256], 250)
                yo3["p23"] = t2
        else:
            if par == 0:
                psE3["p05"] = ps[:, 0:128]
                psE3["p14"] = ps[:, 128:256]
            else:
                t1 = yop.tile([64, 128], F32, tag="yo3", name="yo3_p05", bufs=3)
                t2 = yop.tile([64, 128], F32, tag="yo3", name="yo3_p14", bufs=3)
                bal.copy(t1, ps[:, 0:128], 250)
                bal.copy(t2, ps[:, 128:256], 250)
                yo3["p05"] = t1
                yo3["p14"] = t2
    z1cat3 = zp.tile([64, 3, 128], F32, tag="z3c", name="z1cat3")
    z2cat3 = zp.tile([64, 3, 128], F32, tag="z3c", name="z2cat3")
    for o in ORI:
        bal.q2c((z1cat3, z2cat3), o, psE3[o], yo3[o], 64, 200)
    nc.sync.dma_start(out=yh3[n, ch, 0:3].rearrange("o p w r -> p o (w r)"),
                      in_=z1cat3)
    nc.sync.dma_start(out=yh3[n, ch, 3:6].rearrange("o p w r -> p o (w r)"),
                      in_=z2cat3)


# ----------------------------------------------------------------------------
# Public entry point
# ----------------------------------------------------------------------------

_CACHE = {}


def _get_nc():
    if "nc" not in _CACHE:
        _CACHE["nc"] = build_nc()
    return _CACHE["nc"]


def kernel(x, h0o, h1o, h0a, h0b, h1a, h1b):
    x = np.ascontiguousarray(np.asarray(x), dtype=np.float32)
    mats = build_host_mats(np.asarray(h0o), np.asarray(h1o), np.asarray(h0a),
                           np.asarray(h0b), np.asarray(h1a), np.asarray(h1b))
    nc = _get_nc()

    in_maps = []
    for c in range(N_CORES):
        m = {"x": np.ascontiguousarray(x[c * NS:(c + 1) * NS])}
        m.update(mats)
        in_maps.append(m)

    try:
        res = run_bass_kernel_spmd(nc, in_maps, core_ids=list(range(N_CORES)))
    except Exception:
        # transient device faults (e.g. NRT_EXEC_UNIT_UNRECOVERABLE) recover
        # on retry with a freshly built program
        _CACHE.clear()
        nc = _get_nc()
        res = run_bass_kernel_spmd(nc, in_maps, core_ids=list(range(N_CORES)))

    yl = np.concatenate([res.results[c]["yl"] for c in range(N_CORES)], axis=0)
    yh1 = np.concatenate([res.results[c]["yh1"] for c in range(N_CORES)], axis=0)
    yh2 = np.concatenate([res.results[c]["yh2"] for c in range(N_CORES)], axis=0)
    yh3 = np.concatenate([res.results[c]["yh3"] for c in range(N_CORES)], axis=0)
    return yl, yh1, yh2, yh3


# revision 14
# speedup vs baseline: 2597.4443x; 1.0159x over previous
"""DTCWT forward (J=3) for x:[16,3,512,512] f32 on 8 Trainium2 NeuronCores.

Batch data-parallel: 2 samples (6 images) per core. Every 1-D filtering
stage (level-1 odd-tap col/row filters, level-2/3 q-shift decimated
filters, incl. symmetric extension + tree interleave + sqrt(1/2) scaling)
is a host-precomputed operator matrix, run on the tensor engine as fp32r
matmuls with the image data as the stationary operand, so each stage's
output is transposed relative to its input and H/W stage pairs chain with
no explicit transposes. Moving operands are *windowed* to the band support
(padded to >=256 free for full-rate fp32r). H-stage outputs are split into
even/odd row planes so the q2c butterflies are free-axis stride-2 vector
ops, reading the even-plane straight out of PSUM.
"""

import sys

for _p in ("/opt/trn_rl_repo", "/root/.axon_site/_ro/trn_rl_repo"):
    if _p not in sys.path:
        sys.path.append(_p)

from contextlib import ExitStack

import numpy as np

import concourse.bacc as bacc
import concourse.tile as tile
from concourse import mybir
from concourse.bass_utils import run_bass_kernel_spmd

F32 = mybir.dt.float32
F32R = mybir.dt.float32r
SQRT1_2 = 0.7071067811865476

N_CORES = 8
NS = 2
NC_CH = 3

# orientation slots (z1, z2) per filter pair
SLOT = {"p05": (0, 5), "p14": (1, 4), "p23": (2, 3)}
ORI = ["p05", "p23", "p14"]


# ----------------------------------------------------------------------------
# Host-side operator matrices
# ----------------------------------------------------------------------------

def _refl(idx, n):
    idx = np.asarray(idx)
    idx = np.mod(idx, 2 * n)
    return np.where(idx < n, idx, 2 * n - 1 - idx)


def colfilter_mat(h, n):
    h = np.asarray(h, dtype=np.float64)
    L = h.shape[0]
    m = L // 2
    A = np.zeros((n, n), dtype=np.float64)
    for i in range(n):
        for k in range(L):
            A[i, int(_refl(i + k - m, n))] += h[k]
    return A


def dfilt_mat(fa, fb, n, highpass):
    fa = np.asarray(fa, dtype=np.float64)
    fb = np.asarray(fb, dtype=np.float64)
    m = fa.shape[0]
    farev, fbrev = fa[::-1], fb[::-1]
    n_q = n // 4
    Da = np.zeros((n_q, n), dtype=np.float64)
    Db = np.zeros((n_q, n), dtype=np.float64)
    for i in range(n_q):
        for k in range(m):
            Da[i, int(_refl(4 * i + 2 + 2 * k - m, n))] += farev[k]
            Db[i, int(_refl(4 * i + 3 + 2 * k - m, n))] += fbrev[k]
    first, second = (Db, Da) if highpass else (Da, Db)
    D = np.zeros((n // 2, n), dtype=np.float64)
    D[0::2] = first
    D[1::2] = second
    return D


def _allmats(h0o, h1o, h0a, h0b, h1a, h1b):
    return {
        "C5": colfilter_mat(h0o, 512), "C7": colfilter_mat(h1o, 512),
        "Dlo": dfilt_mat(h0b, h0a, 512, False),
        "Dhi": dfilt_mat(h1b, h1a, 512, True),
        "Dlo3": dfilt_mat(h0b, h0a, 256, False),
        "Dhi3": dfilt_mat(h1b, h1a, 256, True),
    }


def _nzrange(M2):
    nz = np.where(np.abs(M2).sum(axis=0) > 0)[0]
    return int(nz[0]), int(nz[-1]) + 1


def _fitwin(lo, hi, width, limit):
    """Even-aligned window [s, s+width) within [0, limit] covering [lo, hi)."""
    assert hi - lo <= width, (lo, hi, width)
    s = max(0, hi - width)
    s += s % 2  # round up to even so the window still covers hi
    s = min(s, limit - width)
    assert s <= lo and s + width >= hi and 0 <= s <= limit - width and s % 2 == 0
    return s


def build_plan(mats):
    """Window metadata + packed (windowed, scale-folded) moving tensors."""
    s = SQRT1_2
    C5, C7 = mats["C5"], mats["C7"]
    Dlo, Dhi = mats["Dlo"], mats["Dhi"]
    Dlo3, Dhi3 = mats["Dlo3"], mats["Dhi3"]
    W = {}
    P = {}

    # L1-A: per (j, filter f): [Cf_e | Cf_o] cols, shared window width 128
    MA = np.zeros((128, 4, 2, 2, 128), dtype=np.float64)  # [k, j, f, par, x]
    W["A1"] = []
    cov = np.zeros((2, 2, 256), dtype=bool)
    for j in range(4):
        ws = []
        for f, C in enumerate((C5, C7)):
            lo_e, hi_e = _nzrange(C[0::2][:, 128 * j:128 * j + 128].T)
            lo_o, hi_o = _nzrange(C[1::2][:, 128 * j:128 * j + 128].T)
            w = _fitwin(min(lo_e, lo_o), max(hi_e, hi_o), 128, 256)
            ws.append(w)
            for par, Cp in enumerate((C[0::2], C[1::2])):
                MA[:, j, f, par, :] = Cp[w:w + 128, 128 * j:128 * j + 128].T
                cov[f, par, w:w + 128] = True
        W["A1"].append(ws)
    assert cov.all()
    P["MA"] = MA.reshape(128, 4, 512)

    # L1-B: per j: window 256 over 512; variants [C5, s*C5, s*C7]
    MB = np.zeros((128, 4, 3, 256), dtype=np.float64)
    W["B1"] = []
    cov = np.zeros(512, dtype=bool)
    for j in range(4):
        l1, h1 = _nzrange(C5[:, 128 * j:128 * j + 128].T)
        l2, h2 = _nzrange(C7[:, 128 * j:128 * j + 128].T)
        w = _fitwin(min(l1, l2), max(h1, h2), 256, 512)
        W["B1"].append(w)
        MB[:, j, 0, :] = C5[w:w + 256, 128 * j:128 * j + 128].T
        MB[:, j, 1, :] = s * C5[w:w + 256, 128 * j:128 * j + 128].T
        MB[:, j, 2, :] = s * C7[w:w + 256, 128 * j:128 * j + 128].T
        cov[w:w + 256] = True
    assert cov.all()
    P["MB"] = MB.reshape(128, 4, 768)

    # L2-A: per idx=(par,hc): 4 quarters [Dlo_e|Dlo_o|Dhi_e|Dhi_o],
    # shared window width 72 within each 128-quarter
    Dq = [Dlo[0::2], Dlo[1::2], Dhi[0::2], Dhi[1::2]]
    ML2 = np.zeros((128, 4, 4, 72), dtype=np.float64)  # [k, idx, q, x]
    W["A2"] = []
    cov = np.zeros((4, 128), dtype=bool)
    for idx in range(4):
        par, hc = divmod(idx, 2)
        cols = 2 * (128 * hc + np.arange(128)) + par
        rngs = [_nzrange(D[:, cols].T) for D in Dq]
        w = _fitwin(min(r[0] for r in rngs), max(r[1] for r in rngs), 72, 128)
        W["A2"].append(w)
        for q, D in enumerate(Dq):
            ML2[:, idx, q, :] = D[w:w + 72, :][:, cols].T
            cov[q, w:w + 72] = True
    assert cov.all()
    P["ML2"] = ML2.reshape(128, 4, 288)

    # L2-B: per (j, variant v): halves [lo_scale*Dlo | s*Dhi], window 128/half
    ML2B = np.zeros((128, 4, 2, 2, 128), dtype=np.float64)  # [k, j, v, h, x]
    W["B2"] = []
    cov = np.zeros(256, dtype=bool)
    for j in range(4):
        r1 = _nzrange(Dlo[:, 128 * j:128 * j + 128].T)
        r2 = _nzrange(Dhi[:, 128 * j:128 * j + 128].T)
        w = _fitwin(min(r1[0], r2[0]), max(r1[1], r2[1]), 128, 256)
        W["B2"].append(w)
        for v, lo_scale in enumerate((1.0, s)):
            ML2B[:, j, v, 0, :] = lo_scale * Dlo[w:w + 128, 128 * j:128 * j + 128].T
            ML2B[:, j, v, 1, :] = s * Dhi[w:w + 128, 128 * j:128 * j + 128].T
        cov[w:w + 128] = True
    assert cov.all()
    P["ML2B"] = ML2B.reshape(128, 4, 512)

    # L3-A: dense, per input parity
    D3cat = np.vstack([Dlo3[0::2], Dlo3[1::2], Dhi3[0::2], Dhi3[1::2]])
    ML3 = np.zeros((128, 2, 256), dtype=np.float64)
    for par in range(2):
        cols = 2 * np.arange(128) + par
        ML3[:, par, :] = D3cat[:, cols].T
    P["ML3"] = ML3

    # L3-B: dense, per (m, variant)
    ML3B = np.zeros((128, 2, 2, 256), dtype=np.float64)
    for m in range(2):
        sl = slice(128 * m, 128 * m + 128)
        for v, lo_scale in enumerate((1.0, s)):
            ML3B[:, m, v, 0:128] = lo_scale * Dlo3[:, sl].T
            ML3B[:, m, v, 128:256] = s * Dhi3[:, sl].T
    P["ML3B"] = ML3B.reshape(128, 2, 512)

    P = {k: np.ascontiguousarray(v, dtype=np.float32) for k, v in P.items()}
    return W, P


def structural_windows():
    o = np.ones
    return build_plan(_allmats(o(5), o(7), o(10), o(10), o(10), o(10)))[0]


def build_host_mats(h0o, h1o, h0a, h0b, h1a, h1b):
    return build_plan(_allmats(h0o, h1o, h0a, h0b, h1a, h1b))[1]


# ----------------------------------------------------------------------------
# Bass program
# ----------------------------------------------------------------------------

def build_nc(repeat=1):
    Wn = structural_windows()
    nc = bacc.Bacc("TRN2", target_bir_lowering=False)

    x = nc.dram_tensor("x", [NS, NC_CH, 512, 512], F32R, kind="ExternalInput")
    dMA = nc.dram_tensor("MA", [128, 4, 512], F32R, kind="ExternalInput")
    dMB = nc.dram_tensor("MB", [128, 4, 768], F32R, kind="ExternalInput")
    dML2 = nc.dram_tensor("ML2", [128, 4, 288], F32R, kind="ExternalInput")
    dML2B = nc.dram_tensor("ML2B", [128, 4, 512], F32R, kind="ExternalInput")
    dML3 = nc.dram_tensor("ML3", [128, 2, 256], F32R, kind="ExternalInput")
    dML3B = nc.dram_tensor("ML3B", [128, 2, 512], F32R, kind="ExternalInput")

    yl = nc.dram_tensor("yl", [NS, NC_CH, 128, 128], F32, kind="ExternalOutput")
    yh1 = nc.dram_tensor("yh1", [NS, NC_CH, 6, 256, 256, 2], F32, kind="ExternalOutput")
    yh2 = nc.dram_tensor("yh2", [NS, NC_CH, 6, 128, 128, 2], F32, kind="ExternalOutput")
    yh3 = nc.dram_tensor("yh3", [NS, NC_CH, 6, 64, 64, 2], F32, kind="ExternalOutput")

    with tile.TileContext(nc) as tc:
        with ExitStack() as ctx:
            mp = ctx.enter_context(tc.tile_pool(name="mp", bufs=1))
            xp = ctx.enter_context(tc.tile_pool(name="xp", bufs=2))
            ltp = ctx.enter_context(tc.tile_pool(name="ltp", bufs=2))
            lolop = ctx.enter_context(tc.tile_pool(name="lolop", bufs=2))
            yop = ctx.enter_context(tc.tile_pool(name="yop", bufs=2))
            zp = ctx.enter_context(tc.tile_pool(name="zp", bufs=6))
            l2p = ctx.enter_context(tc.tile_pool(name="l2p", bufs=2))
            l3p = ctx.enter_context(tc.tile_pool(name="l3p", bufs=2))
            psp = ctx.enter_context(tc.tile_pool(name="psp", bufs=8, space="PSUM"))

            sMA = mp.tile([128, 4, 512], F32R)
            nc.sync.dma_start(out=sMA, in_=dMA[:, :, :])
            sMB = mp.tile([128, 4, 768], F32R)
            nc.sync.dma_start(out=sMB, in_=dMB[:, :, :])
            sML2 = mp.tile([128, 4, 288], F32R)
            nc.sync.dma_start(out=sML2, in_=dML2[:, :, :])
            sML2B = mp.tile([128, 4, 512], F32R)
            nc.sync.dma_start(out=sML2B, in_=dML2B[:, :, :])
            sML3 = mp.tile([128, 2, 256], F32R)
            nc.sync.dma_start(out=sML3, in_=dML3[:, :, :])
            sML3B = mp.tile([128, 2, 512], F32R)
            nc.sync.dma_start(out=sML3B, in_=dML3B[:, :, :])
            sm = dict(MA=sMA, MB=sMB, ML2=sML2, ML2B=sML2B, ML3=sML3,
                      ML3B=sML3B)
            pools = dict(xp=xp, ltp=ltp, lolop=lolop, yop=yop, zp=zp,
                         l2p=l2p, l3p=l3p, psp=psp)

            bal = EngBal(nc)

            def body():
                for n in range(NS):
                    for ch in range(NC_CH):
                        img_kernel(nc, x, yl, yh1, yh2, yh3, n, ch, sm,
                                   pools, Wn, bal)

            if repeat > 1:
                with tc.For_i(0, repeat, 1):
                    body()
            else:
                body()
    nc.compile()
    return nc


class EngBal:
    """Greedy ACT/DVE load balancer for PSUM->SBUF drains; q2c work is
    charged to DVE so drains flow to whichever engine is lighter."""

    def __init__(self, nc):
        self.nc = nc
        self.act = 0.0
        self.dve = 0.0

    def copy(self, out, in_, cost):
        if self.act <= self.dve:
            self.nc.scalar.copy(out, in_)
            self.act += cost
        else:
            self.nc.vector.tensor_copy(out, in_)
            self.dve += cost

    def q2c(self, zcats, o, E, O, w2, cost):
        """4 butterfly ops for orientation o into the cat z tiles."""
        nc = self.nc
        z1cat, z2cat = zcats
        s1, s2 = SLOT[o]
        a, b = E[:, 0::2], E[:, 1::2]
        c, d = O[:, 0::2], O[:, 1::2]
        z1 = z1cat[:, s1, :]
        z2 = z2cat[:, s2 - 3, :]
        nc.vector.tensor_sub(z1[:, 0::2], a, d)
        nc.vector.tensor_add(z1[:, 1::2], b, c)
        nc.vector.tensor_add(z2[:, 0::2], a, d)
        nc.vector.tensor_sub(z2[:, 1::2], b, c)
        self.dve += 4 * cost


def img_kernel(nc, x, yl, yh1, yh2, yh3, n, ch, sm, pools, Wn, bal):
    xp, ltp, lolop, yop, zp, l2p, l3p, psp = (
        pools["xp"], pools["ltp"], pools["lolop"], pools["yop"], pools["zp"],
        pools["l2p"], pools["l3p"], pools["psp"])

    xt = xp.tile([128, 4, 512], F32R, tag="xt")
    nc.sync.dma_start(out=xt, in_=x[n, ch].rearrange("(t p) w -> p t w", p=128))

    # ---- L1-A -> loT/hiT [128 (W), 4 wblk, 512=(e:256|o:256)]
    loT = ltp.tile([128, 4, 512], F32R, tag="loT")
    hiT = ltp.tile([128, 4, 512], F32R, tag="hiT")
    for m in range(4):
        ps_lo = psp.tile([128, 512], F32, tag="ps", bufs=8)
        ps_hi = psp.tile([128, 512], F32, tag="ps", bufs=8)
        plo = ps_lo.rearrange("p (c x) -> p c x", c=2)
        phi = ps_hi.rearrange("p (c x) -> p c x", c=2)
        for j in range(4):
            st = xt[:, j, 128 * m:128 * m + 128]
            w0 = Wn["A1"][j][0]
            w1 = Wn["A1"][j][1]
            nc.tensor.matmul(plo[:, :, w0:w0 + 128], st,
                             sm["MA"][:, j, 0:256],
                             start=(j == 0), stop=(j == 3))
            nc.tensor.matmul(phi[:, :, w1:w1 + 128], st,
                             sm["MA"][:, j, 256:512],
                             start=(j == 0), stop=(j == 3))
        bal.copy(loT[:, m, :], ps_lo, 730)
        bal.copy(hiT[:, m, :], ps_hi, 730)

    # ---- L1-B (+ q2c L1) per hc
    lolo_e = lolop.tile([128, 2, 512], F32R, tag="lolo_e")
    lolo_o = lolop.tile([128, 2, 512], F32R, tag="lolo_o")
    for hc in range(2):
        psE = {}
        yo = {}
        for srcname, par in (("lo", 0), ("lo", 1), ("hi", 0), ("hi", 1)):
            src = loT if srcname == "lo" else hiT
            off = 256 * par + 128 * hc
            v1 = 0 if srcname == "lo" else 1
            ps1 = psp.tile([128, 512], F32, tag="ps", bufs=8)
            ps2 = psp.tile([128, 512], F32, tag="ps", bufs=8)
            for j in range(4):
                st = src[:, j, off:off + 128]
                w = Wn["B1"][j]
                nc.tensor.matmul(ps1[:, w:w + 256], st,
                                 sm["MB"][:, j, 256 * v1:256 * v1 + 256],
                                 start=(j == 0), stop=(j == 3))
                nc.tensor.matmul(ps2[:, w:w + 256], st,
                                 sm["MB"][:, j, 512:768],
                                 start=(j == 0), stop=(j == 3))
            if srcname == "lo":
                dst = lolo_e if par == 0 else lolo_o
                bal.copy(dst[:, hc, :], ps1, 730)
                if par == 0:
                    psE["p23"] = ps2
                else:
                    t = yop.tile([128, 512], F32, tag="yo", name="yo_p23", bufs=3)
                    bal.copy(t, ps2, 730)
                    yo["p23"] = t
            else:
                if par == 0:
                    psE["p05"] = ps1
                    psE["p14"] = ps2
                else:
                    t1 = yop.tile([128, 512], F32, tag="yo", name="yo_p05", bufs=3)
                    t2 = yop.tile([128, 512], F32, tag="yo", name="yo_p14", bufs=3)
                    bal.copy(t1, ps1, 730)
                    bal.copy(t2, ps2, 730)
                    yo["p05"] = t1
                    yo["p14"] = t2
        z1cat = zp.tile([128, 3, 512], F32, tag="z", name="z1cat")
        z2cat = zp.tile([128, 3, 512], F32, tag="z", name="z2cat")
        for o in ORI:
            bal.q2c((z1cat, z2cat), o, psE[o], yo[o], 256, 533)
        d1 = yh1[n, ch, 0:3].rearrange("o (t p) w r -> t p o (w r)", p=128)
        d2 = yh1[n, ch, 3:6].rearrange("o (t p) w r -> t p o (w r)", p=128)
        nc.sync.dma_start(out=d1[hc], in_=z1cat)
        nc.sync.dma_start(out=d2[hc], in_=z2cat)

    # ---- L2-A -> lt2 [128 (W), 4 wblk, 512=(lo_e|lo_o|hi_e|hi_o)]
    lt2 = l2p.tile([128, 4, 512], F32R, tag="lt2")
    for m in range(4):
        ps = psp.tile([128, 512], F32, tag="ps", bufs=8)
        pq = ps.rearrange("p (q x) -> p q x", q=4)
        for idx in range(4):
            par, hc = divmod(idx, 2)
            src = lolo_e if par == 0 else lolo_o
            w = Wn["A2"][idx]
            nc.tensor.matmul(pq[:, :, w:w + 72],
                             src[:, hc, 128 * m:128 * m + 128],
                             sm["ML2"][:, idx, :],
                             start=(idx == 0), stop=(idx == 3))
        bal.copy(lt2[:, m, :], ps, 730)

    # ---- L2-B (+ q2c L2)
    lolo2_e = l2p.tile([128, 256], F32R, tag="lolo2_e")
    lolo2_o = l2p.tile([128, 256], F32R, tag="lolo2_o")
    psE2 = {}
    yo2 = {}
    for srcname, par in (("lo", 0), ("lo", 1), ("hi", 0), ("hi", 1)):
        off = 256 * (0 if srcname == "lo" else 1) + 128 * par
        v = 0 if srcname == "lo" else 1
        ps = psp.tile([128, 512], F32, tag="ps", bufs=8)
        ph = ps.rearrange("p (h x) -> p h x", h=2)
        for j in range(4):
            w = Wn["B2"][j]
            nc.tensor.matmul(ph[:, :, w:w + 128], lt2[:, j, off:off + 128],
                             sm["ML2B"][:, j, 256 * v:256 * v + 256],
                             start=(j == 0), stop=(j == 3))
        if srcname == "lo":
            dst = lolo2_e if par == 0 else lolo2_o
            bal.copy(dst, ps[:, 0:256], 420)
            if par == 0:
                psE2["p23"] = ps[:, 256:512]
            else:
                t = yop.tile([128, 256], F32, tag="yo2", name="yo2_p23", bufs=3)
                bal.copy(t, ps[:, 256:512], 420)
                yo2["p23"] = t
        else:
            if par == 0:
                psE2["p05"] = ps[:, 0:256]
                psE2["p14"] = ps[:, 256:512]
            else:
                t1 = yop.tile([128, 256], F32, tag="yo2", name="yo2_p05", bufs=3)
                t2 = yop.tile([128, 256], F32, tag="yo2", name="yo2_p14", bufs=3)
                bal.copy(t1, ps[:, 0:256], 420)
                bal.copy(t2, ps[:, 256:512], 420)
                yo2["p05"] = t1
                yo2["p14"] = t2
    z1cat2 = zp.tile([128, 3, 256], F32, tag="z2c", name="z1cat2")
    z2cat2 = zp.tile([128, 3, 256], F32, tag="z2c", name="z2cat2")
    for o in ORI:
        bal.q2c((z1cat2, z2cat2), o, psE2[o], yo2[o], 128, 300)
    nc.sync.dma_start(out=yh2[n, ch, 0:3].rearrange("o p w r -> p o (w r)"),
                      in_=z1cat2)
    nc.sync.dma_start(out=yh2[n, ch, 3:6].rearrange("o p w r -> p o (w r)"),
                      in_=z2cat2)

    # ---- L3-A -> lt3 [128 (W2 chunk), 2 m, 256=(lo_e|lo_o|hi_e|hi_o) x 64]
    lt3 = l3p.tile([128, 2, 256], F32R, tag="lt3")
    for m in range(2):
        ps = psp.tile([128, 256], F32, tag="ps", bufs=8)
        for par in range(2):
            src = lolo2_e if par == 0 else lolo2_o
            nc.tensor.matmul(ps, src[:, 128 * m:128 * m + 128],
                             sm["ML3"][:, par, :],
                             start=(par == 0), stop=(par == 1))
        bal.copy(lt3[:, m, :], ps, 420)

    # ---- L3-B (+ q2c L3, yl)
    yl_ap = yl[n, ch].rearrange("(h par) w -> par h w", par=2)
    psE3 = {}
    yo3 = {}
    for srcname, par in (("lo", 0), ("lo", 1), ("hi", 0), ("hi", 1)):
        off = 128 * (0 if srcname == "lo" else 1) + 64 * par
        v = 0 if srcname == "lo" else 1
        ps = psp.tile([64, 256], F32, tag="ps", bufs=8)
        for m in range(2):
            nc.tensor.matmul(ps, lt3[:, m, off:off + 64],
                             sm["ML3B"][:, m, 256 * v:256 * v + 256],
                             start=(m == 0), stop=(m == 1))
        if srcname == "lo":
            t = l3p.tile([64, 128], F32, tag="lolo3", name="lolo3_%d" % par)
            bal.copy(t, ps[:, 0:128], 250)
            nc.sync.dma_start(out=yl_ap[par], in_=t)
            if par == 0:
                psE3["p23"] = ps[:, 128:256]
            else:
                t2 = yop.tile([64, 128], F32, tag="yo3", name="yo3_p23", bufs=3)
                bal.copy(t2, ps[:, 128:256], 250)
                yo3["p23"] = t2
        else:
            if par == 0:
                psE3["p05"] = ps[:, 0:128]
                psE3["p14"] = ps[:, 128:256]
            else:
                t1 = yop.tile([64, 128], F32, tag="yo3", name="yo3_p05", bufs=3)
                t2 = yop.tile([64, 128], F32, tag="yo3", name="yo3_p14", bufs=3)
                bal.copy(t1, ps[:, 0:128], 250)
                bal.copy(t2, ps[:, 128:256], 250)
                yo3["p05"] = t1
                yo3["p14"] = t2
    z1cat3 = zp.tile([64, 3, 128], F32, tag="z3c", name="z1cat3")
    z2cat3 = zp.tile([64, 3, 128], F32, tag="z3c", name="z2cat3")
    for o in ORI:
        bal.q2c((z1cat3, z2cat3), o, psE3[o], yo3[o], 64, 200)
    nc.sync.dma_start(out=yh3[n, ch, 0:3].rearrange("o p w r -> p o (w r)"),
                      in_=z1cat3)
    nc.sync.dma_start(out=yh3[n, ch, 3:6].rearrange("o p w r -> p o (w r)"),
                      in_=z2cat3)


# ----------------------------------------------------------------------------
# Public entry point
# ----------------------------------------------------------------------------

_CACHE = {}


def _get_nc():
    if "nc" not in _CACHE:
        _CACHE["nc"] = build_nc()
    return _CACHE["nc"]


def kernel(x, h0o, h1o, h0a, h0b, h1a, h1b):
    x = np.ascontiguousarray(np.asarray(x), dtype=np.float32)
    mats = build_host_mats(np.asarray(h0o), np.asarray(h1o), np.asarray(h0a),
                           np.asarray(h0b), np.asarray(h1a), np.asarray(h1b))
    nc = _get_nc()

    in_maps = []
    for c in range(N_CORES):
        m = {"x": np.ascontiguousarray(x[c * NS:(c + 1) * NS])}
        m.update(mats)
        in_maps.append(m)

    try:
        res = run_bass_kernel_spmd(nc, in_maps, core_ids=list(range(N_CORES)))
    except Exception:
        # transient device faults (e.g. NRT_EXEC_UNIT_UNRECOVERABLE) recover
        # on retry with a freshly built program
        _CACHE.clear()
        nc = _get_nc()
        res = run_bass_kernel_spmd(nc, in_maps, core_ids=list(range(N_CORES)))

    yl = np.concatenate([res.results[c]["yl"] for c in range(N_CORES)], axis=0)
    yh1 = np.concatenate([res.results[c]["yh1"] for c in range(N_CORES)], axis=0)
    yh2 = np.concatenate([res.results[c]["yh2"] for c in range(N_CORES)], axis=0)
    yh3 = np.concatenate([res.results[c]["yh3"] for c in range(N_CORES)], axis=0)
    return yl, yh1, yh2, yh3


# revision 15
# speedup vs baseline: 2847.4872x; 1.0963x over previous
"""DTCWT forward (J=3) for x:[16,3,512,512] f32 on 8 Trainium2 NeuronCores.

Batch data-parallel: 2 samples (6 images) per core. Every 1-D filtering
stage (level-1 odd-tap col/row filters, level-2/3 q-shift decimated
filters, incl. symmetric extension + tree interleave + sqrt(1/2) scaling)
is a host-precomputed operator matrix, run on the tensor engine as fp32r
matmuls with the image data as the stationary operand, so each stage's
output is transposed relative to its input and H/W stage pairs chain with
no explicit transposes. Moving operands are *windowed* to the band support
(padded to >=256 free for full-rate fp32r). H-stage outputs are split into
even/odd row planes so the q2c butterflies are free-axis stride-2 vector
ops, reading the even-plane straight out of PSUM.
"""

import sys

for _p in ("/opt/trn_rl_repo", "/root/.axon_site/_ro/trn_rl_repo"):
    if _p not in sys.path:
        sys.path.append(_p)

from contextlib import ExitStack

import numpy as np

import concourse.bacc as bacc
import concourse.tile as tile
from concourse import mybir
from concourse.bass_utils import run_bass_kernel_spmd

F32 = mybir.dt.float32
F32R = mybir.dt.float32r
SQRT1_2 = 0.7071067811865476

N_CORES = 8
NS = 2
NC_CH = 3

# orientation slots (z1, z2) per filter pair
SLOT = {"p05": (0, 5), "p14": (1, 4), "p23": (2, 3)}
ORI = ["p05", "p23", "p14"]


# ----------------------------------------------------------------------------
# Host-side operator matrices
# ----------------------------------------------------------------------------

def _refl(idx, n):
    idx = np.asarray(idx)
    idx = np.mod(idx, 2 * n)
    return np.where(idx < n, idx, 2 * n - 1 - idx)


def colfilter_mat(h, n):
    h = np.asarray(h, dtype=np.float64)
    L = h.shape[0]
    m = L // 2
    A = np.zeros((n, n), dtype=np.float64)
    for i in range(n):
        for k in range(L):
            A[i, int(_refl(i + k - m, n))] += h[k]
    return A


def dfilt_mat(fa, fb, n, highpass):
    fa = np.asarray(fa, dtype=np.float64)
    fb = np.asarray(fb, dtype=np.float64)
    m = fa.shape[0]
    farev, fbrev = fa[::-1], fb[::-1]
    n_q = n // 4
    Da = np.zeros((n_q, n), dtype=np.float64)
    Db = np.zeros((n_q, n), dtype=np.float64)
    for i in range(n_q):
        for k in range(m):
            Da[i, int(_refl(4 * i + 2 + 2 * k - m, n))] += farev[k]
            Db[i, int(_refl(4 * i + 3 + 2 * k - m, n))] += fbrev[k]
    first, second = (Db, Da) if highpass else (Da, Db)
    D = np.zeros((n // 2, n), dtype=np.float64)
    D[0::2] = first
    D[1::2] = second
    return D


def _allmats(h0o, h1o, h0a, h0b, h1a, h1b):
    return {
        "C5": colfilter_mat(h0o, 512), "C7": colfilter_mat(h1o, 512),
        "Dlo": dfilt_mat(h0b, h0a, 512, False),
        "Dhi": dfilt_mat(h1b, h1a, 512, True),
        "Dlo3": dfilt_mat(h0b, h0a, 256, False),
        "Dhi3": dfilt_mat(h1b, h1a, 256, True),
    }


def _nzrange(M2):
    nz = np.where(np.abs(M2).sum(axis=0) > 0)[0]
    return int(nz[0]), int(nz[-1]) + 1


def _fitwin(lo, hi, width, limit):
    """Even-aligned window [s, s+width) within [0, limit] covering [lo, hi)."""
    assert hi - lo <= width, (lo, hi, width)
    s = max(0, hi - width)
    s += s % 2  # round up to even so the window still covers hi
    s = min(s, limit - width)
    assert s <= lo and s + width >= hi and 0 <= s <= limit - width and s % 2 == 0
    return s


def build_plan(mats):
    """Window metadata + packed (windowed, scale-folded) moving tensors."""
    s = SQRT1_2
    C5, C7 = mats["C5"], mats["C7"]
    Dlo, Dhi = mats["Dlo"], mats["Dhi"]
    Dlo3, Dhi3 = mats["Dlo3"], mats["Dhi3"]
    W = {}
    P = {}

    # L1-A: per (j, filter f): [Cf_e | Cf_o] cols, shared window width 128
    MA = np.zeros((128, 4, 2, 2, 128), dtype=np.float64)  # [k, j, f, par, x]
    W["A1"] = []
    cov = np.zeros((2, 2, 256), dtype=bool)
    for j in range(4):
        ws = []
        for f, C in enumerate((C5, C7)):
            lo_e, hi_e = _nzrange(C[0::2][:, 128 * j:128 * j + 128].T)
            lo_o, hi_o = _nzrange(C[1::2][:, 128 * j:128 * j + 128].T)
            w = _fitwin(min(lo_e, lo_o), max(hi_e, hi_o), 128, 256)
            ws.append(w)
            for par, Cp in enumerate((C[0::2], C[1::2])):
                MA[:, j, f, par, :] = Cp[w:w + 128, 128 * j:128 * j + 128].T
                cov[f, par, w:w + 128] = True
        W["A1"].append(ws)
    assert cov.all()
    P["MA"] = MA.reshape(128, 4, 512)

    # L1-B: per j: window 256 over 512; variants [C5, s*C5, s*C7]
    MB = np.zeros((128, 4, 3, 256), dtype=np.float64)
    W["B1"] = []
    cov = np.zeros(512, dtype=bool)
    for j in range(4):
        l1, h1 = _nzrange(C5[:, 128 * j:128 * j + 128].T)
        l2, h2 = _nzrange(C7[:, 128 * j:128 * j + 128].T)
        w = _fitwin(min(l1, l2), max(h1, h2), 256, 512)
        W["B1"].append(w)
        MB[:, j, 0, :] = C5[w:w + 256, 128 * j:128 * j + 128].T
        MB[:, j, 1, :] = s * C5[w:w + 256, 128 * j:128 * j + 128].T
        MB[:, j, 2, :] = s * C7[w:w + 256, 128 * j:128 * j + 128].T
        cov[w:w + 256] = True
    assert cov.all()
    P["MB"] = MB.reshape(128, 4, 768)

    # L2-A: per idx=(par,hc): 4 quarters [Dlo_e|Dlo_o|Dhi_e|Dhi_o],
    # shared window width 72 within each 128-quarter
    Dq = [Dlo[0::2], Dlo[1::2], Dhi[0::2], Dhi[1::2]]
    ML2 = np.zeros((128, 4, 4, 72), dtype=np.float64)  # [k, idx, q, x]
    W["A2"] = []
    cov = np.zeros((4, 128), dtype=bool)
    for idx in range(4):
        par, hc = divmod(idx, 2)
        cols = 2 * (128 * hc + np.arange(128)) + par
        rngs = [_nzrange(D[:, cols].T) for D in Dq]
        w = _fitwin(min(r[0] for r in rngs), max(r[1] for r in rngs), 72, 128)
        W["A2"].append(w)
        for q, D in enumerate(Dq):
            ML2[:, idx, q, :] = D[w:w + 72, :][:, cols].T
            cov[q, w:w + 72] = True
    assert cov.all()
    P["ML2"] = ML2.reshape(128, 4, 288)

    # L2-B: per (j, variant v): halves [lo_scale*Dlo | s*Dhi], window 128/half
    ML2B = np.zeros((128, 4, 2, 2, 128), dtype=np.float64)  # [k, j, v, h, x]
    W["B2"] = []
    cov = np.zeros(256, dtype=bool)
    for j in range(4):
        r1 = _nzrange(Dlo[:, 128 * j:128 * j + 128].T)
        r2 = _nzrange(Dhi[:, 128 * j:128 * j + 128].T)
        w = _fitwin(min(r1[0], r2[0]), max(r1[1], r2[1]), 128, 256)
        W["B2"].append(w)
        for v, lo_scale in enumerate((1.0, s)):
            ML2B[:, j, v, 0, :] = lo_scale * Dlo[w:w + 128, 128 * j:128 * j + 128].T
            ML2B[:, j, v, 1, :] = s * Dhi[w:w + 128, 128 * j:128 * j + 128].T
        cov[w:w + 128] = True
    assert cov.all()
    P["ML2B"] = ML2B.reshape(128, 4, 512)

    # L3-A: dense, per input parity
    D3cat = np.vstack([Dlo3[0::2], Dlo3[1::2], Dhi3[0::2], Dhi3[1::2]])
    ML3 = np.zeros((128, 2, 256), dtype=np.float64)
    for par in range(2):
        cols = 2 * np.arange(128) + par
        ML3[:, par, :] = D3cat[:, cols].T
    P["ML3"] = ML3

    # L3-B: dense, per (m, variant)
    ML3B = np.zeros((128, 2, 2, 256), dtype=np.float64)
    for m in range(2):
        sl = slice(128 * m, 128 * m + 128)
        for v, lo_scale in enumerate((1.0, s)):
            ML3B[:, m, v, 0:128] = lo_scale * Dlo3[:, sl].T
            ML3B[:, m, v, 128:256] = s * Dhi3[:, sl].T
    P["ML3B"] = ML3B.reshape(128, 2, 512)

    P = {k: np.ascontiguousarray(v, dtype=np.float32) for k, v in P.items()}
    return W, P


def structural_windows():
    o = np.ones
    return build_plan(_allmats(o(5), o(7), o(10), o(10), o(10), o(10)))[0]


def build_host_mats(h0o, h1o, h0a, h0b, h1a, h1b):
    return build_plan(_allmats(h0o, h1o, h0a, h0b, h1a, h1b))[1]


# ----------------------------------------------------------------------------
# Bass program
# ----------------------------------------------------------------------------

def build_nc(repeat=1):
    Wn = structural_windows()
    nc = bacc.Bacc("TRN2", target_bir_lowering=False)

    x = nc.dram_tensor("x", [NS, NC_CH, 512, 512], F32R, kind="ExternalInput")
    dMA = nc.dram_tensor("MA", [128, 4, 512], F32R, kind="ExternalInput")
    dMB = nc.dram_tensor("MB", [128, 4, 768], F32R, kind="ExternalInput")
    dML2 = nc.dram_tensor("ML2", [128, 4, 288], F32R, kind="ExternalInput")
    dML2B = nc.dram_tensor("ML2B", [128, 4, 512], F32R, kind="ExternalInput")
    dML3 = nc.dram_tensor("ML3", [128, 2, 256], F32R, kind="ExternalInput")
    dML3B = nc.dram_tensor("ML3B", [128, 2, 512], F32R, kind="ExternalInput")

    yl = nc.dram_tensor("yl", [NS, NC_CH, 128, 128], F32, kind="ExternalOutput")
    yh1 = nc.dram_tensor("yh1", [NS, NC_CH, 6, 256, 256, 2], F32, kind="ExternalOutput")
    yh2 = nc.dram_tensor("yh2", [NS, NC_CH, 6, 128, 128, 2], F32, kind="ExternalOutput")
    yh3 = nc.dram_tensor("yh3", [NS, NC_CH, 6, 64, 64, 2], F32, kind="ExternalOutput")

    with tile.TileContext(nc) as tc:
        with ExitStack() as ctx:
            mp = ctx.enter_context(tc.tile_pool(name="mp", bufs=1))
            xp = ctx.enter_context(tc.tile_pool(name="xp", bufs=2))
            ltp = ctx.enter_context(tc.tile_pool(name="ltp", bufs=2))
            lolop = ctx.enter_context(tc.tile_pool(name="lolop", bufs=2))
            yop = ctx.enter_context(tc.tile_pool(name="yop", bufs=2))
            zp = ctx.enter_context(tc.tile_pool(name="zp", bufs=6))
            l2p = ctx.enter_context(tc.tile_pool(name="l2p", bufs=2))
            l3p = ctx.enter_context(tc.tile_pool(name="l3p", bufs=2))
            psp = ctx.enter_context(tc.tile_pool(name="psp", bufs=8, space="PSUM"))

            sMA = mp.tile([128, 4, 512], F32R)
            nc.sync.dma_start(out=sMA, in_=dMA[:, :, :])
            sMB = mp.tile([128, 4, 768], F32R)
            nc.sync.dma_start(out=sMB, in_=dMB[:, :, :])
            sML2 = mp.tile([128, 4, 288], F32R)
            nc.sync.dma_start(out=sML2, in_=dML2[:, :, :])
            sML2B = mp.tile([128, 4, 512], F32R)
            nc.sync.dma_start(out=sML2B, in_=dML2B[:, :, :])
            sML3 = mp.tile([128, 2, 256], F32R)
            nc.sync.dma_start(out=sML3, in_=dML3[:, :, :])
            sML3B = mp.tile([128, 2, 512], F32R)
            nc.sync.dma_start(out=sML3B, in_=dML3B[:, :, :])
            sm = dict(MA=sMA, MB=sMB, ML2=sML2, ML2B=sML2B, ML3=sML3,
                      ML3B=sML3B)
            pools = dict(xp=xp, ltp=ltp, lolop=lolop, yop=yop, zp=zp,
                         l2p=l2p, l3p=l3p, psp=psp)

            bal = EngBal(nc)

            def body():
                for n in range(NS):
                    for ch in range(NC_CH):
                        img_kernel(nc, x, yl, yh1, yh2, yh3, n, ch, sm,
                                   pools, Wn, bal)

            if repeat > 1:
                with tc.For_i(0, repeat, 1):
                    body()
            else:
                body()
    nc.compile()
    return nc


class EngBal:
    """Greedy ACT/DVE load balancer for PSUM->SBUF drains; q2c work is
    charged to DVE so drains flow to whichever engine is lighter."""

    def __init__(self, nc):
        self.nc = nc
        self.act = 0.0
        self.dve = 0.0

    def copy(self, out, in_, cost):
        if self.act <= self.dve:
            self.nc.scalar.copy(out, in_)
            self.act += cost
        else:
            self.nc.vector.tensor_copy(out, in_)
            self.dve += cost

    def q2c(self, zcats, o, E, O, w2, cost):
        """4 butterfly ops for orientation o into the cat z tiles."""
        nc = self.nc
        z1cat, z2cat = zcats
        s1, s2 = SLOT[o]
        a, b = E[:, 0::2], E[:, 1::2]
        c, d = O[:, 0::2], O[:, 1::2]
        z1 = z1cat[:, s1, :]
        z2 = z2cat[:, s2 - 3, :]
        nc.vector.tensor_sub(z1[:, 0::2], a, d)
        nc.vector.tensor_add(z1[:, 1::2], b, c)
        nc.vector.tensor_add(z2[:, 0::2], a, d)
        nc.vector.tensor_sub(z2[:, 1::2], b, c)
        self.dve += 4 * cost


def img_kernel(nc, x, yl, yh1, yh2, yh3, n, ch, sm, pools, Wn, bal):
    xp, ltp, lolop, yop, zp, l2p, l3p, psp = (
        pools["xp"], pools["ltp"], pools["lolop"], pools["yop"], pools["zp"],
        pools["l2p"], pools["l3p"], pools["psp"])

    xt = xp.tile([128, 4, 512], F32R, tag="xt")
    nc.sync.dma_start(out=xt, in_=x[n, ch].rearrange("(t p) w -> p t w", p=128))

    # ---- L1-A -> loT/hiT [128 (W), 4 wblk, 512=(e:256|o:256)]
    loT = ltp.tile([128, 4, 512], F32R, tag="loT")
    hiT = ltp.tile([128, 4, 512], F32R, tag="hiT")
    for m in range(4):
        ps_lo = psp.tile([128, 512], F32, tag="ps", bufs=8)
        ps_hi = psp.tile([128, 512], F32, tag="ps", bufs=8)
        plo = ps_lo.rearrange("p (c x) -> p c x", c=2)
        phi = ps_hi.rearrange("p (c x) -> p c x", c=2)
        for j in range(4):
            st = xt[:, j, 128 * m:128 * m + 128]
            w0 = Wn["A1"][j][0]
            w1 = Wn["A1"][j][1]
            nc.tensor.matmul(plo[:, :, w0:w0 + 128], st,
                             sm["MA"][:, j, 0:256],
                             start=(j == 0), stop=(j == 3))
            nc.tensor.matmul(phi[:, :, w1:w1 + 128], st,
                             sm["MA"][:, j, 256:512],
                             start=(j == 0), stop=(j == 3))
        bal.copy(loT[:, m, :], ps_lo, 730)
        bal.copy(hiT[:, m, :], ps_hi, 730)

    # ---- L1-B (+ q2c L1) per hc
    lolo_e = lolop.tile([128, 2, 512], F32R, tag="lolo_e")
    lolo_o = lolop.tile([128, 2, 512], F32R, tag="lolo_o")
    for hc in range(2):
        z1cat = zp.tile([128, 3, 512], F32, tag="z", name="z1cat")
        z2cat = zp.tile([128, 3, 512], F32, tag="z", name="z2cat")

        def l1b_group(srcname, par):
            src = loT if srcname == "lo" else hiT
            off = 256 * par + 128 * hc
            v1 = 0 if srcname == "lo" else 1
            ps1 = psp.tile([128, 512], F32, tag="ps", name="ps", bufs=8)
            ps2 = psp.tile([128, 512], F32, tag="ps", name="ps", bufs=8)
            for j in range(4):
                st = src[:, j, off:off + 128]
                w = Wn["B1"][j]
                nc.tensor.matmul(ps1[:, w:w + 256], st,
                                 sm["MB"][:, j, 256 * v1:256 * v1 + 256],
                                 start=(j == 0), stop=(j == 3))
                nc.tensor.matmul(ps2[:, w:w + 256], st,
                                 sm["MB"][:, j, 512:768],
                                 start=(j == 0), stop=(j == 3))
            return ps1, ps2

        ps1, ps2 = l1b_group("lo", 0)
        bal.copy(lolo_e[:, hc, :], ps1, 730)
        psE_p23 = ps2
        ps1, ps2 = l1b_group("lo", 1)
        bal.copy(lolo_o[:, hc, :], ps1, 730)
        yo_p23 = yop.tile([128, 512], F32, tag="yo", name="yo_p23", bufs=3)
        bal.copy(yo_p23, ps2, 730)
        bal.q2c((z1cat, z2cat), "p23", psE_p23, yo_p23, 256, 533)

        ps1_e, ps2_e = l1b_group("hi", 0)
        ps1, ps2 = l1b_group("hi", 1)
        yo_p05 = yop.tile([128, 512], F32, tag="yo", name="yo_p05", bufs=3)
        yo_p14 = yop.tile([128, 512], F32, tag="yo", name="yo_p14", bufs=3)
        bal.copy(yo_p05, ps1, 730)
        bal.q2c((z1cat, z2cat), "p05", ps1_e, yo_p05, 256, 533)
        bal.copy(yo_p14, ps2, 730)
        bal.q2c((z1cat, z2cat), "p14", ps2_e, yo_p14, 256, 533)

        d1 = yh1[n, ch, 0:3].rearrange("o (t p) w r -> t p o (w r)", p=128)
        d2 = yh1[n, ch, 3:6].rearrange("o (t p) w r -> t p o (w r)", p=128)
        nc.sync.dma_start(out=d1[hc], in_=z1cat)
        nc.sync.dma_start(out=d2[hc], in_=z2cat)

    # ---- L2-A -> lt2 [128 (W), 4 wblk, 512=(lo_e|lo_o|hi_e|hi_o)]
    lt2 = l2p.tile([128, 4, 512], F32R, tag="lt2")
    for m in range(4):
        ps = psp.tile([128, 512], F32, tag="ps", bufs=8)
        pq = ps.rearrange("p (q x) -> p q x", q=4)
        for idx in range(4):
            par, hc = divmod(idx, 2)
            src = lolo_e if par == 0 else lolo_o
            w = Wn["A2"][idx]
            nc.tensor.matmul(pq[:, :, w:w + 72],
                             src[:, hc, 128 * m:128 * m + 128],
                             sm["ML2"][:, idx, :],
                             start=(idx == 0), stop=(idx == 3))
        bal.copy(lt2[:, m, :], ps, 730)

    # ---- L2-B (+ q2c L2)
    lolo2_e = l2p.tile([128, 256], F32R, tag="lolo2_e")
    lolo2_o = l2p.tile([128, 256], F32R, tag="lolo2_o")
    psE2 = {}
    yo2 = {}
    for srcname, par in (("lo", 0), ("lo", 1), ("hi", 0), ("hi", 1)):
        off = 256 * (0 if srcname == "lo" else 1) + 128 * par
        v = 0 if srcname == "lo" else 1
        ps = psp.tile([128, 512], F32, tag="ps", bufs=8)
        ph = ps.rearrange("p (h x) -> p h x", h=2)
        for j in range(4):
            w = Wn["B2"][j]
            nc.tensor.matmul(ph[:, :, w:w + 128], lt2[:, j, off:off + 128],
                             sm["ML2B"][:, j, 256 * v:256 * v + 256],
                             start=(j == 0), stop=(j == 3))
        if srcname == "lo":
            dst = lolo2_e if par == 0 else lolo2_o
            bal.copy(dst, ps[:, 0:256], 420)
            if par == 0:
                psE2["p23"] = ps[:, 256:512]
            else:
                t = yop.tile([128, 256], F32, tag="yo2", name="yo2_p23", bufs=3)
                bal.copy(t, ps[:, 256:512], 420)
                yo2["p23"] = t
        else:
            if par == 0:
                psE2["p05"] = ps[:, 0:256]
                psE2["p14"] = ps[:, 256:512]
            else:
                t1 = yop.tile([128, 256], F32, tag="yo2", name="yo2_p05", bufs=3)
                t2 = yop.tile([128, 256], F32, tag="yo2", name="yo2_p14", bufs=3)
                bal.copy(t1, ps[:, 0:256], 420)
                bal.copy(t2, ps[:, 256:512], 420)
                yo2["p05"] = t1
                yo2["p14"] = t2
    z1cat2 = zp.tile([128, 3, 256], F32, tag="z2c", name="z1cat2")
    z2cat2 = zp.tile([128, 3, 256], F32, tag="z2c", name="z2cat2")
    for o in ORI:
        bal.q2c((z1cat2, z2cat2), o, psE2[o], yo2[o], 128, 300)
    nc.sync.dma_start(out=yh2[n, ch, 0:3].rearrange("o p w r -> p o (w r)"),
                      in_=z1cat2)
    nc.sync.dma_start(out=yh2[n, ch, 3:6].rearrange("o p w r -> p o (w r)"),
                      in_=z2cat2)

    # ---- L3-A -> lt3 [128 (W2 chunk), 2 m, 256=(lo_e|lo_o|hi_e|hi_o) x 64]
    lt3 = l3p.tile([128, 2, 256], F32R, tag="lt3")
    for m in range(2):
        ps = psp.tile([128, 256], F32, tag="ps", bufs=8)
        for par in range(2):
            src = lolo2_e if par == 0 else lolo2_o
            nc.tensor.matmul(ps, src[:, 128 * m:128 * m + 128],
                             sm["ML3"][:, par, :],
                             start=(par == 0), stop=(par == 1))
        bal.copy(lt3[:, m, :], ps, 420)

    # ---- L3-B (+ q2c L3, yl)
    yl_ap = yl[n, ch].rearrange("(h par) w -> par h w", par=2)
    psE3 = {}
    yo3 = {}
    for srcname, par in (("lo", 0), ("lo", 1), ("hi", 0), ("hi", 1)):
        off = 128 * (0 if srcname == "lo" else 1) + 64 * par
        v = 0 if srcname == "lo" else 1
        ps = psp.tile([64, 256], F32, tag="ps", bufs=8)
        for m in range(2):
            nc.tensor.matmul(ps, lt3[:, m, off:off + 64],
                             sm["ML3B"][:, m, 256 * v:256 * v + 256],
                             start=(m == 0), stop=(m == 1))
        if srcname == "lo":
            t = l3p.tile([64, 128], F32, tag="lolo3", name="lolo3_%d" % par)
            bal.copy(t, ps[:, 0:128], 250)
            nc.sync.dma_start(out=yl_ap[par], in_=t)
            if par == 0:
                psE3["p23"] = ps[:, 128:256]
            else:
                t2 = yop.tile([64, 128], F32, tag="yo3", name="yo3_p23", bufs=3)
                bal.copy(t2, ps[:, 128:256], 250)
                yo3["p23"] = t2
        else:
            if par == 0:
                psE3["p05"] = ps[:, 0:128]
                psE3["p14"] = ps[:, 128:256]
            else:
                t1 = yop.tile([64, 128], F32, tag="yo3", name="yo3_p05", bufs=3)
                t2 = yop.tile([64, 128], F32, tag="yo3", name="yo3_p14", bufs=3)
                bal.copy(t1, ps[:, 0:128], 250)
                bal.copy(t2, ps[:, 128:256], 250)
                yo3["p05"] = t1
                yo3["p14"] = t2
    z1cat3 = zp.tile([64, 3, 128], F32, tag="z3c", name="z1cat3")
    z2cat3 = zp.tile([64, 3, 128], F32, tag="z3c", name="z2cat3")
    for o in ORI:
        bal.q2c((z1cat3, z2cat3), o, psE3[o], yo3[o], 64, 200)
    nc.sync.dma_start(out=yh3[n, ch, 0:3].rearrange("o p w r -> p o (w r)"),
                      in_=z1cat3)
    nc.sync.dma_start(out=yh3[n, ch, 3:6].rearrange("o p w r -> p o (w r)"),
                      in_=z2cat3)


# ----------------------------------------------------------------------------
# Public entry point
# ----------------------------------------------------------------------------

_CACHE = {}


def _get_nc():
    if "nc" not in _CACHE:
        _CACHE["nc"] = build_nc()
    return _CACHE["nc"]


def kernel(x, h0o, h1o, h0a, h0b, h1a, h1b):
    x = np.ascontiguousarray(np.asarray(x), dtype=np.float32)
    mats = build_host_mats(np.asarray(h0o), np.asarray(h1o), np.asarray(h0a),
                           np.asarray(h0b), np.asarray(h1a), np.asarray(h1b))
    nc = _get_nc()

    in_maps = []
    for c in range(N_CORES):
        m = {"x": np.ascontiguousarray(x[c * NS:(c + 1) * NS])}
        m.update(mats)
        in_maps.append(m)

    try:
        res = run_bass_kernel_spmd(nc, in_maps, core_ids=list(range(N_CORES)))
    except Exception:
        # transient device faults (e.g. NRT_EXEC_UNIT_UNRECOVERABLE) recover
        # on retry with a freshly built program
        _CACHE.clear()
        nc = _get_nc()
        res = run_bass_kernel_spmd(nc, in_maps, core_ids=list(range(N_CORES)))

    yl = np.concatenate([res.results[c]["yl"] for c in range(N_CORES)], axis=0)
    yh1 = np.concatenate([res.results[c]["yh1"] for c in range(N_CORES)], axis=0)
    yh2 = np.concatenate([res.results[c]["yh2"] for c in range(N_CORES)], axis=0)
    yh3 = np.concatenate([res.results[c]["yh3"] for c in range(N_CORES)], axis=0)
    return yl, yh1, yh2, yh3
